# revision 53
# baseline (speedup 1.0000x reference)
"""Trainium2 Bass kernel for a 2-layer dense GCN (NodeEncoder).

    out = adj @ relu(adj @ (x@W1) + b1) @ W2 + b2
    N=16384, F_IN=512, HID=1024, OUT=256, adj dense [N, N] fp32.

Algorithm (reassociated to nearly halve layer-1 FLOPs and drop the big
s1 AllGather):  relu(adj @ (x@W1)) == relu((adj@x) @ W1), so per core
(adj row-partitioned, 2048 rows each):

  P1:    yT_c   = x8^T @ adjN8_c^T                     [512, 2048]  (= N*y^T)
  small: hT_c   = relu(yT_c^T @ (W1/N) + b1)^T         [1024, 2048] bf16
         s2_c   = h_c @ W2                             [2048, 256]
         quantized to fp8 * 256 for the gather.
  AG:    s2q    = AllGather(s2q_c)  (4 chunks of 128KB, overlapped)
  P2:    out2T_c = (adjN8_c @ s2q)^T / (N*256) + b2    [256, 2048] fp32

Big matmuls run in fp8-e4m3 DoubleRow (K=256/instr); small ones bf16.
Simulated end-to-end rel err ~1.55e-2 vs fp32 reference (tol 2e-2),
dominated by the fp8 quantization of x.
"""

import numpy as np
import ml_dtypes

import concourse.bass as bass
import concourse.mybir as mybir
import concourse.tile as tile
from concourse.bass_utils import run_bass_kernel_spmd
from concourse.tile_sem_assignment import N_PROCS
from concourse.vector_clock import ScopedClock, VectorClock

# ---------------------------------------------------------------------------
# Workaround: the walrus build in this container caps the number of sync-wait
# commands per instruction at ONE.  Tile's kernel-tail drain aggregates one
# wait per logical processor; split it into a chain of single-wait drains.
# Excess waits on regular instructions are hoisted onto no-ops.
# ---------------------------------------------------------------------------


def _drain_and_barrier_split(self, tick_clock, wait_clock):
    gc = tick_clock.global_clock
    for p in range(N_PROCS):
        partial = VectorClock([gc[q] if q == p else 0 for q in range(N_PROCS)])
        d = self.nc.sync.nop(nofuse=True)
        wait_clock.add_sem_waits(d.ins, ScopedClock({None: partial}))
    self.nc.sync.drain()

    self.nc.all_engine_barrier()
    assert self.sems is not None
    popped = self.nc._tile_sem_poison_stack.pop()
    assert popped is self._sem_poison
    self.nc.clear_and_free_semaphores(list(self.sems.allocated().values()))
    self.nc.all_engine_barrier()


tile.TileContext._drain_and_barrier = _drain_and_barrier_split

_MAX_WAITS = 1


def _split_excess_waits(nc):
    ctr = 0
    for f in nc.m.functions:
        for bb in f.blocks:
            out = []
            changed = False
            for inst in bb.instructions:
                si = inst.sync_info
                waits = list(si.on_wait) if si is not None and si.on_wait else []
                if len(waits) > _MAX_WAITS:
                    changed = True
                    keep, excess = waits[: _MAX_WAITS], waits[_MAX_WAITS :]
                    for i in range(0, len(excess), _MAX_WAITS):
                        ctr += 1
                        nop = mybir.InstNoOp(name=f"I-waitnop-{ctr}")
                        nop.engine = inst.engine
                        nop.sync_info = mybir.SyncInfo(
                            on_wait=excess[i : i + _MAX_WAITS], on_update=[]
                        )
                        out.append(nop)
                    si.on_wait = keep
                out.append(inst)
            if changed:
                bb.instructions = out
    return ctr


def _elide_redundant_ldweights(nc):
    """Drop an InstLdweights that reloads the same weights AP as the previous
    surviving one with only plain matmuls/no-ops in between (the PE keeps the
    stationary operand across matmuls; walrus emits one LDWEIGHTS per MATMUL)."""
    n_elided = 0
    for f in nc.m.functions:
        for bb in f.blocks:
            out = []
            last_w = None
            changed = False
            for inst in bb.instructions:
                nm = type(inst).__name__
                if nm == "InstLdweights":
                    si = inst.sync_info
                    clean = not (si and (si.on_wait or si.on_update))
                    w = repr(inst.ins[0])
                    if clean and last_w == w:
                        n_elided += 1
                        changed = True
                        continue
                    last_w = w if clean else None
                elif nm == "InstMatmult":
                    if getattr(inst, "is_transpose", False):
                        last_w = None
                elif nm == "InstNoOp":
                    pass
                else:
                    last_w = None
                out.append(inst)
            if changed:
                bb.instructions = out
    return n_elided


NCORES = 8
N = 16384
SH = N // NCORES  # 2048 adj rows per core
F = 512
HID = 1024
OUT = 256
S2SCALE = 256.0  # s2 is gathered as fp8 of 256*s2

BF16 = mybir.dt.bfloat16
F32 = mybir.dt.float32
FP8 = mybir.dt.float8e4
DR = mybir.MatmulPerfMode.DoubleRow

_built = None


def build():
    nc = bass.Bass()

    # adjU row r = kb*128 + p (k = kb*256 + kk*128 + p global col of adjT_c),
    # col = kk*2048 + i (i = local row of the adj shard), values N*adj in fp8.
    # P1 reads the 1024-wide i-half slices, P2 reads full rows.
    adjU = nc.declare_dram_parameter("adjU", [8192, 2 * SH], FP8, isOutput=False)
    # xP row = kb*128 + p, col = kk*512 + j
    xP = nc.declare_dram_parameter("xP", [8192, 2 * F], FP8, isOutput=False)
    w1n = nc.declare_dram_parameter("w1n", [F, HID], BF16, isOutput=False)  # W1/N
    w2 = nc.declare_dram_parameter("w2", [HID, OUT], BF16, isOutput=False)
    b1T = nc.declare_dram_parameter("b1T", [128, HID // 128], F32, isOutput=False)
    b2T = nc.declare_dram_parameter("b2T", [128, OUT // 128], F32, isOutput=False)
    out2T = nc.declare_dram_parameter("out2T", [OUT, SH], F32, isOutput=True)

    rg = [list(range(NCORES))]

    def allgather(inp, outp):
        return nc.gpsimd.collective_compute(
            "AllGather",
            mybir.AluOpType.bypass,
            replica_groups=rg,
            ins=[inp.opt()],
            outs=[outp.opt()],
        )

    with tile.TileContext(nc) as tc:
        with (
            tc.tile_pool(name="const", bufs=1) as constp,
            tc.tile_pool(name="psum", bufs=8, space="PSUM") as psum,
            tc.tile_pool(name="dram", bufs=1, space="DRAM") as dram,
            tc.tile_pool(name="adj", bufs=6) as adjp,
            tc.tile_pool(name="small", bufs=4) as smallp,
        ):
            # ---- constants / resident tensors ----
            # consts go on the ACT dma queue so the SP queue starts with the
            # x/adj tiles that gate the first matmul
            w1t = constp.tile([128, 4, HID], BF16)  # [j%128, jj, hid]
            nc.scalar.dma_start(w1t[:], w1n[:].rearrange("(jj p) h -> p jj h", p=128))
            w2t = constp.tile([128, 8, OUT], BF16)  # [hid%128, hh, j2]
            nc.scalar.dma_start(w2t[:], w2[:].rearrange("(hh p) o -> p hh o", p=128))
            b1t = constp.tile([128, 8], F32)
            nc.scalar.dma_start(b1t[:], b1T[:])
            b2t = constp.tile([128, 2], F32)
            nc.scalar.dma_start(b2t[:], b2T[:])
            # x streams per k-block (re-read in each half) -- cheaper than
            # keeping all 8MB resident; the freed SBUF holds more adj cache
            xP_r = xP[:].rearrange("(kb p) (kk j) -> p kb kk j", p=128, kk=2)

            # results kept in SBUF
            yT = constp.tile([128, 4, SH], BF16)  # [j%128, jj, i] = N*y
            hT = constp.tile([128, 8, SH], BF16)  # [hid%128, hh, i]

            # AllGather staging: chunk q covers local rows [512q, 512q+512)
            # laid out [p, kbl, kk, j2] (row = kbl*256 + kk*128 + p).
            ag_in = [dram.tile([128, 1024], FP8, name=f"agi{q}") for q in range(4)]
            ag_out = [
                dram.tile([NCORES * 128, 1024], FP8, addr_space="Shared", name=f"ago{q}")
                for q in range(4)
            ]

            adjU_r = adjU[:].rearrange("(kb p) (kk i) -> p kb kk i", p=128, kk=2)

            # P2 is DMA-bandwidth-bound: keep adj k-blocks loaded during P1
            # resident in SBUF so P2 skips their reload (saves 10MB of the
            # ~36MB P2 stream).  Spread across the AG-arrival groups.
            CACHE_KBS = tuple(range(0, 40, 2))
            adj_cache = {}

            for H in range(2):
                # ---- P1 half H: psY[j, i-1024-half] += x8^T adjC8 ----
                psY = [
                    psum.tile([128, 512], F32, tag="ps", name=f"psY{H}{t}")
                    for t in range(8)
                ]  # tile t = (jj, b): jj*2 + b; holds i-chunks 2b, 2b+1
                for kb in range(64):
                    first = H == 0 and kb == 0
                    xt = smallp.tile([128, 2, F], FP8, tag="xt", bufs=8)
                    if first:
                        # chunked so the very first matmul's deps land early
                        for jj in range(4):
                            nc.sync.dma_start(
                                xt[:, :, jj * 128 : (jj + 1) * 128],
                                xP_r[:, kb, :, jj * 128 : (jj + 1) * 128],
                            )
                    else:
                        nc.sync.dma_start(xt[:], xP_r[:, kb])
                    if kb in CACHE_KBS:
                        at = constp.tile([128, 2, 1024], FP8, name=f"ac_{H}_{kb}")
                        adj_cache[(H, kb)] = at
                    else:
                        at = adjp.tile(
                            [128, 2, 1024], FP8, tag="adjt", name=f"a1_{H}_{kb}"
                        )
                    src = adjU_r[:, kb, :, H * 1024 : (H + 1) * 1024]
                    if first:
                        for c in range(4):
                            nc.sync.dma_start(
                                at[:, :, c * 256 : (c + 1) * 256],
                                src[:, :, c * 256 : (c + 1) * 256],
                            )
                    else:
                        nc.sync.dma_start(at[:], src)
                    for jj in range(4):
                        lhs = xt[:, :, jj * 128 : (jj + 1) * 128]
                        for c in range(4):  # i-chunk of 256 within the half
                            nc.tensor.matmul(
                                psY[jj * 2 + c // 2][:, (c % 2) * 256 : (c % 2) * 256 + 256],
                                lhs,
                                at[:, :, c * 256 : (c + 1) * 256],
                                start=(kb == 0 and c % 2 == 0),
                                stop=(kb == 63 and c % 2 == 1),
                                perf_mode=DR,
                            )
                # drain psY -> yT (bf16) on the vector engine (ACT is busier)
                for jj in range(4):
                    for b in range(2):
                        nc.vector.tensor_copy(
                            yT[:, jj, H * 1024 + b * 512 : H * 1024 + b * 512 + 512],
                            psY[jj * 2 + b][:],
                        )

                # ---- supportT + relu: hT = relu(W1n^T yT + b1) ----
                # hh-groups of 4 with i-width 1024: each stationary W1 block
                # feeds two 512-wide matmuls, halving LDWEIGHTS count.
                for hg in range(2):
                    i0 = H * 1024
                    psS = [
                        psum.tile([128, 512], F32, tag="ps", name=f"psS{H}{hg}{t}")
                        for t in range(8)
                    ]  # t = hh4*2 + qq
                    for hh4 in range(4):
                        hh = hg * 4 + hh4
                        for jj in range(4):
                            for qq in range(2):
                                nc.tensor.matmul(
                                    psS[hh4 * 2 + qq][:],
                                    w1t[:, jj, hh * 128 : (hh + 1) * 128],
                                    yT[:, jj, i0 + qq * 512 : i0 + qq * 512 + 512],
                                    start=(jj == 0),
                                    stop=(jj == 3),
                                )
                    # relu drains split ACT/DVE: the first s2 matmul group is
                    # paced by these, so halve the chain latency
                    for hh4 in range(4):
                        hh = hg * 4 + hh4
                        for qq in range(2):
                            dst = hT[:, hh, i0 + qq * 512 : i0 + qq * 512 + 512]
                            src = psS[hh4 * 2 + qq][:]
                            if qq == 1:
                                nc.vector.tensor_scalar(
                                    dst,
                                    src,
                                    b1t[:, hh : hh + 1],
                                    0.0,
                                    op0=mybir.AluOpType.add,
                                    op1=mybir.AluOpType.max,
                                )
                            else:
                                nc.scalar.activation(
                                    dst,
                                    src,
                                    mybir.ActivationFunctionType.Relu,
                                    bias=b1t[:, hh : hh + 1],
                                )

                # ---- s2 = h @ W2, quantized fp8*256, staged for AG ----
                # psum/AG column order (j2t, kk, jp) so P2's stationary load
                # is a plain contiguous copy.
                for qq in range(2):
                    q = H * 2 + qq  # global chunk id
                    for kbl in range(2):
                        ps2 = psum.tile(
                            [128, 2, 2, 128], F32, tag="ps", name=f"ps2{q}{kbl}"
                        )
                        for kk in range(2):
                            i0 = q * 512 + kbl * 256 + kk * 128
                            for hh in range(8):
                                nc.tensor.matmul(
                                    ps2[:, :, kk, :],
                                    hT[:, hh, i0 : i0 + 128],
                                    w2t[:, hh, :],
                                    start=(hh == 0 and kk == 0),
                                    stop=(hh == 7 and kk == 1),
                                )
                        s2q = smallp.tile([128, 512], FP8, tag="s2q", bufs=2)
                        nc.scalar.activation(
                            s2q[:],
                            ps2[:].rearrange("p a b j -> p (a b j)"),
                            mybir.ActivationFunctionType.Copy,
                            scale=S2SCALE,
                        )
                        nc.scalar.dma_start(
                            ag_in[q][:, kbl * 512 : kbl * 512 + 512], s2q[:]
                        )
                    allgather(ag_in[q], ag_out[q])

            # ---- P2: out2T = (adjU8 @ s2q)^T / (N*256) + b2 ----
            psD = [
                psum.tile([128, 512], F32, tag="ps", name=f"psD{t}") for t in range(8)
            ]  # tile t = (j2, cb): j2*4 + cb; holds i-chunks 2cb, 2cb+1
            # SBUF-cached k-blocks saved for a bank-staggered tail: the last
            # 4 k-blocks are emitted bank-by-bank so each psum bank stops
            # early and its drain+store pipelines behind the remaining MMs.
            TAIL_KBS = [14, 22, 30, 38]
            kb_order = [
                8 * c + 2 * q + t for q in range(4) for c in range(NCORES) for t in range(2)
            ]
            kb_order = [kb for kb in kb_order if kb not in TAIL_KBS]
            for ki, kb in enumerate(kb_order):
                c, rem = kb // 8, kb % 8
                q, t = rem // 2, rem % 2
                if kb in CACHE_KBS:
                    at = None  # rhs comes from the SBUF-cached P1 tiles
                else:
                    at = adjp.tile([128, 2, 2048], FP8, tag="adjt", name=f"a2_{kb}")
                    nc.sync.dma_start(at[:], adjU_r[:, kb])
                st = smallp.tile([128, 2, 2, 128], FP8, tag="st", bufs=4, name=f"st{kb}")
                # ag_out rows c*128+p, cols t*512 + j2t*256 + kk*128 + jp
                nc.sync.dma_start(
                    st[:].rearrange("p a b j -> p (a b j)"),
                    ag_out[q][c * 128 : (c + 1) * 128, t * 512 : (t + 1) * 512],
                )
                for j2 in range(2):
                    lhs = st[:, j2]
                    for c8 in range(8):
                        if at is not None:
                            rhs = at[:, :, c8 * 256 : (c8 + 1) * 256]
                        else:
                            cc = c8 % 4
                            rhs = adj_cache[(c8 // 4, kb)][
                                :, :, cc * 256 : (cc + 1) * 256
                            ]
                        nc.tensor.matmul(
                            psD[j2 * 4 + c8 // 2][:, (c8 % 2) * 256 : (c8 % 2) * 256 + 256],
                            lhs,
                            rhs,
                            start=(ki == 0 and c8 % 2 == 0),
                            stop=False,
                            perf_mode=DR,
                        )
            # bank-staggered tail over SBUF-resident k-blocks: each bank stops
            # ~1us apart and its drain+store pipelines behind later banks' MMs
            st_tail = {}
            for kb in TAIL_KBS:
                c, rem = kb // 8, kb % 8
                q, t = rem // 2, rem % 2
                stt = smallp.tile([128, 2, 2, 128], FP8, tag="st", bufs=4, name=f"stT{kb}")
                nc.sync.dma_start(
                    stt[:].rearrange("p a b j -> p (a b j)"),
                    ag_out[q][c * 128 : (c + 1) * 128, t * 512 : (t + 1) * 512],
                )
                st_tail[kb] = stt
            for t8 in range(8):
                j2, cb = t8 // 4, t8 % 4
                for n, kb in enumerate(TAIL_KBS):
                    lhs = st_tail[kb][:, j2]
                    for c8 in (2 * cb, 2 * cb + 1):
                        cc = c8 % 4
                        rhs = adj_cache[(c8 // 4, kb)][:, :, cc * 256 : (cc + 1) * 256]
                        nc.tensor.matmul(
                            psD[t8][:, (c8 % 2) * 256 : (c8 % 2) * 256 + 256],
                            lhs,
                            rhs,
                            start=False,
                            stop=(n == len(TAIL_KBS) - 1 and c8 % 2 == 1),
                            perf_mode=DR,
                        )
                ot = smallp.tile([128, 512], F32, tag="ot", bufs=4)
                if cb % 2 == 0:
                    nc.scalar.activation(
                        ot[:],
                        psD[t8][:],
                        mybir.ActivationFunctionType.Identity,
                        bias=b2t[:, j2 : j2 + 1],
                        scale=1.0 / (N * S2SCALE),
                    )
                    nc.scalar.dma_start(
                        out2T[j2 * 128 : (j2 + 1) * 128, cb * 512 : (cb + 1) * 512],
                        ot[:],
                    )
                else:
                    nc.vector.tensor_scalar(
                        ot[:],
                        psD[t8][:],
                        1.0 / (N * S2SCALE),
                        b2t[:, j2 : j2 + 1],
                        op0=mybir.AluOpType.mult,
                        op1=mybir.AluOpType.add,
                    )
                    nc.gpsimd.dma_start(
                        out2T[j2 * 128 : (j2 + 1) * 128, cb * 512 : (cb + 1) * 512],
                        ot[:],
                    )

    _elide_redundant_ldweights(nc)
    _split_excess_waits(nc)
    return nc


def _prep_inputs(x, adj, W1, b1, W2, b2):
    bf = ml_dtypes.bfloat16
    f8 = ml_dtypes.float8_e4m3fn

    u = adj * np.float32(N)  # exact: adj was u/N with N a power of two
    u8 = u.astype(f8)
    x8 = x.astype(f8)
    b1T = np.ascontiguousarray(b1.reshape(HID // 128, 128).T).astype(np.float32)
    b2T = np.ascontiguousarray(b2.reshape(OUT // 128, 128).T).astype(np.float32)
    w1n = (W1 / np.float32(N)).astype(bf)
    w2b = W2.astype(bf)
    # xP[kb*128+p, kk*512+j] = x8[kb*256+kk*128+p, j]
    xP = np.ascontiguousarray(
        x8.reshape(64, 2, 128, F).transpose(0, 2, 1, 3).reshape(8192, 2 * F)
    )

    def adj_layout(a8, rows):
        # out[kb*128+p, kk*2048+i] = a8[rows][i, kb*256+kk*128+p]
        blk = a8[rows, :].reshape(SH, 64, 2, 128)  # [i, kb, kk, p]
        return np.ascontiguousarray(
            blk.transpose(1, 3, 2, 0).reshape(8192, 2 * SH)
        )

    in_maps = []
    for c in range(NCORES):
        rows = slice(c * SH, (c + 1) * SH)
        in_maps.append(
            {
                "adjU": adj_layout(u8, rows),
                "xP": xP,
                "w1n": w1n,
                "w2": w2b,
                "b1T": b1T,
                "b2T": b2T,
            }
        )
    return in_maps


def _run(inputs, trace=False):
    global _built
    if _built is None:
        _built = build()
    in_maps = _prep_inputs(**inputs)
    r = run_bass_kernel_spmd(_built, in_maps, list(range(NCORES)), trace=trace)
    out = np.empty([N, OUT], np.float32)
    for c in range(NCORES):
        out[c * SH : (c + 1) * SH, :] = r.results[c]["out2T"].T
    return out, r


def kernel(x, adj, W1, b1, W2, b2):
    out, _ = _run(dict(x=x, adj=adj, W1=W1, b1=b1, W2=W2, b2=b2))
    return out


# revision 57
# speedup vs baseline: 1.0034x; 1.0034x over previous
"""Trainium2 Bass kernel for a 2-layer dense GCN (NodeEncoder).

    out = adj @ relu(adj @ (x@W1) + b1) @ W2 + b2
    N=16384, F_IN=512, HID=1024, OUT=256, adj dense [N, N] fp32.

Algorithm (reassociated to nearly halve layer-1 FLOPs and drop the big
s1 AllGather):  relu(adj @ (x@W1)) == relu((adj@x) @ W1), so per core
(adj row-partitioned, 2048 rows each):

  P1:    yT_c   = x8^T @ adjN8_c^T                     [512, 2048]  (= N*y^T)
  small: hT_c   = relu(yT_c^T @ (W1/N) + b1)^T         [1024, 2048] bf16
         s2_c   = h_c @ W2                             [2048, 256]
         quantized to fp8 * 256 for the gather.
  AG:    s2q    = AllGather(s2q_c)  (4 chunks of 128KB, overlapped)
  P2:    out2T_c = (adjN8_c @ s2q)^T / (N*256) + b2    [256, 2048] fp32

Big matmuls run in fp8-e4m3 DoubleRow (K=256/instr); small ones bf16.
Simulated end-to-end rel err ~1.55e-2 vs fp32 reference (tol 2e-2),
dominated by the fp8 quantization of x.
"""

import numpy as np
import ml_dtypes

import concourse.bass as bass
import concourse.mybir as mybir
import concourse.tile as tile
from concourse.bass_utils import run_bass_kernel_spmd
from concourse.tile_sem_assignment import N_PROCS
from concourse.vector_clock import ScopedClock, VectorClock

# ---------------------------------------------------------------------------
# Workaround: the walrus build in this container caps the number of sync-wait
# commands per instruction at ONE.  Tile's kernel-tail drain aggregates one
# wait per logical processor; split it into a chain of single-wait drains.
# Excess waits on regular instructions are hoisted onto no-ops.
# ---------------------------------------------------------------------------


def _drain_and_barrier_split(self, tick_clock, wait_clock):
    gc = tick_clock.global_clock
    for p in range(N_PROCS):
        partial = VectorClock([gc[q] if q == p else 0 for q in range(N_PROCS)])
        d = self.nc.sync.nop(nofuse=True)
        wait_clock.add_sem_waits(d.ins, ScopedClock({None: partial}))
    self.nc.sync.drain()

    self.nc.all_engine_barrier()
    assert self.sems is not None
    popped = self.nc._tile_sem_poison_stack.pop()
    assert popped is self._sem_poison
    self.nc.clear_and_free_semaphores(list(self.sems.allocated().values()))
    self.nc.all_engine_barrier()


tile.TileContext._drain_and_barrier = _drain_and_barrier_split

_MAX_WAITS = 1


def _split_excess_waits(nc):
    ctr = 0
    for f in nc.m.functions:
        for bb in f.blocks:
            out = []
            changed = False
            for inst in bb.instructions:
                si = inst.sync_info
                waits = list(si.on_wait) if si is not None and si.on_wait else []
                if len(waits) > _MAX_WAITS:
                    changed = True
                    keep, excess = waits[: _MAX_WAITS], waits[_MAX_WAITS :]
                    for i in range(0, len(excess), _MAX_WAITS):
                        ctr += 1
                        nop = mybir.InstNoOp(name=f"I-waitnop-{ctr}")
                        nop.engine = inst.engine
                        nop.sync_info = mybir.SyncInfo(
                            on_wait=excess[i : i + _MAX_WAITS], on_update=[]
                        )
                        out.append(nop)
                    si.on_wait = keep
                out.append(inst)
            if changed:
                bb.instructions = out
    return ctr


def _elide_redundant_ldweights(nc):
    """Drop an InstLdweights that reloads the same weights AP as the previous
    surviving one with only plain matmuls/no-ops in between (the PE keeps the
    stationary operand across matmuls; walrus emits one LDWEIGHTS per MATMUL)."""
    n_elided = 0
    for f in nc.m.functions:
        for bb in f.blocks:
            out = []
            last_w = None
            changed = False
            for inst in bb.instructions:
                nm = type(inst).__name__
                if nm == "InstLdweights":
                    si = inst.sync_info
                    clean = not (si and (si.on_wait or si.on_update))
                    w = repr(inst.ins[0])
                    if clean and last_w == w:
                        n_elided += 1
                        changed = True
                        continue
                    last_w = w if clean else None
                elif nm == "InstMatmult":
                    if getattr(inst, "is_transpose", False):
                        last_w = None
                elif nm == "InstNoOp":
                    pass
                else:
                    last_w = None
                out.append(inst)
            if changed:
                bb.instructions = out
    return n_elided


NCORES = 8
N = 16384
SH = N // NCORES  # 2048 adj rows per core
F = 512
HID = 1024
OUT = 256
S2SCALE = 256.0  # s2 is gathered as fp8 of 256*s2

BF16 = mybir.dt.bfloat16
F32 = mybir.dt.float32
FP8 = mybir.dt.float8e4
DR = mybir.MatmulPerfMode.DoubleRow

_built = None


def build():
    nc = bass.Bass()

    # adjU row r = kb*128 + p (k = kb*256 + kk*128 + p global col of adjT_c),
    # col = kk*2048 + i (i = local row of the adj shard), values N*adj in fp8.
    # P1 reads the 1024-wide i-half slices, P2 reads full rows.
    adjU = nc.declare_dram_parameter("adjU", [8192, 2 * SH], FP8, isOutput=False)
    # xP row = kb*128 + p, col = kk*512 + j
    xP = nc.declare_dram_parameter("xP", [8192, 2 * F], FP8, isOutput=False)
    w1n = nc.declare_dram_parameter("w1n", [F, HID], BF16, isOutput=False)  # W1/N
    w2 = nc.declare_dram_parameter("w2", [HID, OUT], BF16, isOutput=False)
    b1T = nc.declare_dram_parameter("b1T", [128, HID // 128], F32, isOutput=False)
    b2T = nc.declare_dram_parameter("b2T", [128, OUT // 128], F32, isOutput=False)
    out2T = nc.declare_dram_parameter("out2T", [OUT, SH], F32, isOutput=True)

    rg = [list(range(NCORES))]

    def allgather(inp, outp):
        return nc.gpsimd.collective_compute(
            "AllGather",
            mybir.AluOpType.bypass,
            replica_groups=rg,
            ins=[inp.opt()],
            outs=[outp.opt()],
        )

    with tile.TileContext(nc) as tc:
        with (
            tc.tile_pool(name="const", bufs=1) as constp,
            tc.tile_pool(name="psum", bufs=8, space="PSUM") as psum,
            tc.tile_pool(name="dram", bufs=1, space="DRAM") as dram,
            tc.tile_pool(name="adj", bufs=6) as adjp,
            tc.tile_pool(name="small", bufs=4) as smallp,
        ):
            # ---- constants / resident tensors ----
            # consts go on the ACT dma queue so the SP queue starts with the
            # x/adj tiles that gate the first matmul
            w1t = constp.tile([128, 4, HID], BF16)  # [j%128, jj, hid]
            nc.scalar.dma_start(w1t[:], w1n[:].rearrange("(jj p) h -> p jj h", p=128))
            w2t = constp.tile([128, 8, OUT], BF16)  # [hid%128, hh, j2]
            nc.scalar.dma_start(w2t[:], w2[:].rearrange("(hh p) o -> p hh o", p=128))
            b1t = constp.tile([128, 8], F32)
            nc.scalar.dma_start(b1t[:], b1T[:])
            b2t = constp.tile([128, 2], F32)
            nc.scalar.dma_start(b2t[:], b2T[:])
            # x streams per k-block (re-read in each half) -- cheaper than
            # keeping all 8MB resident; the freed SBUF holds more adj cache
            xP_r = xP[:].rearrange("(kb p) (kk j) -> p kb kk j", p=128, kk=2)

            # results kept in SBUF
            yT = constp.tile([128, 4, SH], BF16)  # [j%128, jj, i] = N*y
            hT = constp.tile([128, 8, SH], BF16)  # [hid%128, hh, i]

            # AllGather staging: chunk q covers local rows [512q, 512q+512)
            # laid out [p, kbl, kk, j2] (row = kbl*256 + kk*128 + p).
            ag_in = [dram.tile([128, 1024], FP8, name=f"agi{q}") for q in range(4)]
            ag_out = [
                dram.tile([NCORES * 128, 1024], FP8, addr_space="Shared", name=f"ago{q}")
                for q in range(4)
            ]

            adjU_r = adjU[:].rearrange("(kb p) (kk i) -> p kb kk i", p=128, kk=2)

            # P2 is DMA-bandwidth-bound: keep adj k-blocks loaded during P1
            # resident in SBUF so P2 skips their reload (saves 10MB of the
            # ~36MB P2 stream).  Spread across the AG-arrival groups.
            CACHE_KBS = tuple(range(0, 40, 2))
            adj_cache = {}

            for H in range(2):
                # ---- P1 half H: psY[j, i-1024-half] += x8^T adjC8 ----
                psY = [
                    psum.tile([128, 512], F32, tag="ps", name=f"psY{H}{t}")
                    for t in range(8)
                ]  # tile t = (jj, b): jj*2 + b; holds i-chunks 2b, 2b+1
                for kb in range(64):
                    xt = smallp.tile([128, 2, F], FP8, tag="xt", bufs=8)
                    nc.sync.dma_start(xt[:], xP_r[:, kb])
                    if kb in CACHE_KBS:
                        at = constp.tile([128, 2, 1024], FP8, name=f"ac_{H}_{kb}")
                        adj_cache[(H, kb)] = at
                    else:
                        at = adjp.tile(
                            [128, 2, 1024], FP8, tag="adjt", name=f"a1_{H}_{kb}"
                        )
                    nc.sync.dma_start(
                        at[:], adjU_r[:, kb, :, H * 1024 : (H + 1) * 1024]
                    )
                    for jj in range(4):
                        lhs = xt[:, :, jj * 128 : (jj + 1) * 128]
                        for c in range(4):  # i-chunk of 256 within the half
                            nc.tensor.matmul(
                                psY[jj * 2 + c // 2][:, (c % 2) * 256 : (c % 2) * 256 + 256],
                                lhs,
                                at[:, :, c * 256 : (c + 1) * 256],
                                start=(kb == 0 and c % 2 == 0),
                                stop=(kb == 63 and c % 2 == 1),
                                perf_mode=DR,
                            )
                # drain psY -> yT (bf16) on the vector engine (ACT is busier)
                for jj in range(4):
                    for b in range(2):
                        nc.vector.tensor_copy(
                            yT[:, jj, H * 1024 + b * 512 : H * 1024 + b * 512 + 512],
                            psY[jj * 2 + b][:],
                        )

                # ---- supportT + relu: hT = relu(W1n^T yT + b1) ----
                # hh-groups of 4 with i-width 1024: each stationary W1 block
                # feeds two 512-wide matmuls, halving LDWEIGHTS count.
                for hg in range(2):
                    i0 = H * 1024
                    psS = [
                        psum.tile([128, 512], F32, tag="ps", name=f"psS{H}{hg}{t}")
                        for t in range(8)
                    ]  # t = hh4*2 + qq
                    for hh4 in range(4):
                        hh = hg * 4 + hh4
                        for jj in range(4):
                            for qq in range(2):
                                nc.tensor.matmul(
                                    psS[hh4 * 2 + qq][:],
                                    w1t[:, jj, hh * 128 : (hh + 1) * 128],
                                    yT[:, jj, i0 + qq * 512 : i0 + qq * 512 + 512],
                                    start=(jj == 0),
                                    stop=(jj == 3),
                                )
                    # relu drains split ACT/DVE: the first s2 matmul group is
                    # paced by these, so halve the chain latency
                    for hh4 in range(4):
                        hh = hg * 4 + hh4
                        for qq in range(2):
                            dst = hT[:, hh, i0 + qq * 512 : i0 + qq * 512 + 512]
                            src = psS[hh4 * 2 + qq][:]
                            if qq == 1:
                                nc.vector.tensor_scalar(
                                    dst,
                                    src,
                                    b1t[:, hh : hh + 1],
                                    0.0,
                                    op0=mybir.AluOpType.add,
                                    op1=mybir.AluOpType.max,
                                )
                            else:
                                nc.scalar.activation(
                                    dst,
                                    src,
                                    mybir.ActivationFunctionType.Relu,
                                    bias=b1t[:, hh : hh + 1],
                                )

                # ---- s2 = h @ W2, quantized fp8*256, staged for AG ----
                # psum/AG column order (j2t, kk, jp) so P2's stationary load
                # is a plain contiguous copy.
                for qq in range(2):
                    q = H * 2 + qq  # global chunk id
                    for kbl in range(2):
                        ps2 = psum.tile(
                            [128, 2, 2, 128], F32, tag="ps", name=f"ps2{q}{kbl}"
                        )
                        for kk in range(2):
                            i0 = q * 512 + kbl * 256 + kk * 128
                            for hh in range(8):
                                nc.tensor.matmul(
                                    ps2[:, :, kk, :],
                                    hT[:, hh, i0 : i0 + 128],
                                    w2t[:, hh, :],
                                    start=(hh == 0 and kk == 0),
                                    stop=(hh == 7 and kk == 1),
                                )
                        s2q = smallp.tile([128, 512], FP8, tag="s2q", bufs=2)
                        nc.scalar.activation(
                            s2q[:],
                            ps2[:].rearrange("p a b j -> p (a b j)"),
                            mybir.ActivationFunctionType.Copy,
                            scale=S2SCALE,
                        )
                        nc.scalar.dma_start(
                            ag_in[q][:, kbl * 512 : kbl * 512 + 512], s2q[:]
                        )
                    allgather(ag_in[q], ag_out[q])

            # ---- P2: out2T = (adjU8 @ s2q)^T / (N*256) + b2 ----
            psD = [
                psum.tile([128, 512], F32, tag="ps", name=f"psD{t}") for t in range(8)
            ]  # tile t = (j2, cb): j2*4 + cb; holds i-chunks 2cb, 2cb+1
            kb_order = [
                8 * c + 2 * q + t for q in range(4) for c in range(NCORES) for t in range(2)
            ]
            for ki, kb in enumerate(kb_order):
                c, rem = kb // 8, kb % 8
                q, t = rem // 2, rem % 2
                if kb in CACHE_KBS:
                    at = None  # rhs comes from the SBUF-cached P1 tiles
                else:
                    at = adjp.tile([128, 2, 2048], FP8, tag="adjt", name=f"a2_{kb}")
                    nc.sync.dma_start(at[:], adjU_r[:, kb])
                st = smallp.tile([128, 2, 2, 128], FP8, tag="st", bufs=4, name=f"st{kb}")
                # ag_out rows c*128+p, cols t*512 + j2t*256 + kk*128 + jp
                nc.sync.dma_start(
                    st[:].rearrange("p a b j -> p (a b j)"),
                    ag_out[q][c * 128 : (c + 1) * 128, t * 512 : (t + 1) * 512],
                )
                for j2 in range(2):
                    lhs = st[:, j2]
                    for c8 in range(8):
                        if at is not None:
                            rhs = at[:, :, c8 * 256 : (c8 + 1) * 256]
                        else:
                            cc = c8 % 4
                            rhs = adj_cache[(c8 // 4, kb)][
                                :, :, cc * 256 : (cc + 1) * 256
                            ]
                        nc.tensor.matmul(
                            psD[j2 * 4 + c8 // 2][:, (c8 % 2) * 256 : (c8 % 2) * 256 + 256],
                            lhs,
                            rhs,
                            start=(ki == 0 and c8 % 2 == 0),
                            stop=(ki == 63 and c8 % 2 == 1),
                            perf_mode=DR,
                        )
            # final drain split across ACT and DVE so the tail is ~2x shorter
            for j2 in range(2):
                for cb in range(4):
                    ot = smallp.tile([128, 512], F32, tag="ot", bufs=4)
                    if cb % 2 == 0:
                        nc.scalar.activation(
                            ot[:],
                            psD[j2 * 4 + cb][:],
                            mybir.ActivationFunctionType.Identity,
                            bias=b2t[:, j2 : j2 + 1],
                            scale=1.0 / (N * S2SCALE),
                        )
                        nc.scalar.dma_start(
                            out2T[j2 * 128 : (j2 + 1) * 128, cb * 512 : (cb + 1) * 512],
                            ot[:],
                        )
                    else:
                        nc.vector.tensor_scalar(
                            ot[:],
                            psD[j2 * 4 + cb][:],
                            1.0 / (N * S2SCALE),
                            b2t[:, j2 : j2 + 1],
                            op0=mybir.AluOpType.mult,
                            op1=mybir.AluOpType.add,
                        )
                        nc.gpsimd.dma_start(
                            out2T[j2 * 128 : (j2 + 1) * 128, cb * 512 : (cb + 1) * 512],
                            ot[:],
                        )

    _elide_redundant_ldweights(nc)
    _split_excess_waits(nc)
    return nc


def _prep_inputs(x, adj, W1, b1, W2, b2):
    bf = ml_dtypes.bfloat16
    f8 = ml_dtypes.float8_e4m3fn

    u = adj * np.float32(N)  # exact: adj was u/N with N a power of two
    u8 = u.astype(f8)
    x8 = x.astype(f8)
    b1T = np.ascontiguousarray(b1.reshape(HID // 128, 128).T).astype(np.float32)
    b2T = np.ascontiguousarray(b2.reshape(OUT // 128, 128).T).astype(np.float32)
    w1n = (W1 / np.float32(N)).astype(bf)
    w2b = W2.astype(bf)
    # xP[kb*128+p, kk*512+j] = x8[kb*256+kk*128+p, j]
    xP = np.ascontiguousarray(
        x8.reshape(64, 2, 128, F).transpose(0, 2, 1, 3).reshape(8192, 2 * F)
    )

    def adj_layout(a8, rows):
        # out[kb*128+p, kk*2048+i] = a8[rows][i, kb*256+kk*128+p]
        blk = a8[rows, :].reshape(SH, 64, 2, 128)  # [i, kb, kk, p]
        return np.ascontiguousarray(
            blk.transpose(1, 3, 2, 0).reshape(8192, 2 * SH)
        )

    in_maps = []
    for c in range(NCORES):
        rows = slice(c * SH, (c + 1) * SH)
        in_maps.append(
            {
                "adjU": adj_layout(u8, rows),
                "xP": xP,
                "w1n": w1n,
                "w2": w2b,
                "b1T": b1T,
                "b2T": b2T,
            }
        )
    return in_maps


def _run(inputs, trace=False):
    global _built
    if _built is None:
        _built = build()
    in_maps = _prep_inputs(**inputs)
    r = run_bass_kernel_spmd(_built, in_maps, list(range(NCORES)), trace=trace)
    out = np.empty([N, OUT], np.float32)
    for c in range(NCORES):
        out[c * SH : (c + 1) * SH, :] = r.results[c]["out2T"].T
    return out, r


def kernel(x, adj, W1, b1, W2, b2):
    out, _ = _run(dict(x=x, adj=adj, W1=W1, b1=b1, W2=W2, b2=b2))
    return out


# revision 58
# speedup vs baseline: 1.0119x; 1.0085x over previous
"""Trainium2 Bass kernel for a 2-layer dense GCN (NodeEncoder).

    out = adj @ relu(adj @ (x@W1) + b1) @ W2 + b2
    N=16384, F_IN=512, HID=1024, OUT=256, adj dense [N, N] fp32.

Algorithm (reassociated to nearly halve layer-1 FLOPs and drop the big
s1 AllGather):  relu(adj @ (x@W1)) == relu((adj@x) @ W1), so per core
(adj row-partitioned, 2048 rows each):

  P1:    yT_c   = x8^T @ adjN8_c^T                     [512, 2048]  (= N*y^T)
  small: hT_c   = relu(yT_c^T @ (W1/N) + b1)^T         [1024, 2048] bf16
         s2_c   = h_c @ W2                             [2048, 256]
         quantized to fp8 * 256 for the gather.
  AG:    s2q    = AllGather(s2q_c)  (4 chunks of 128KB, overlapped)
  P2:    out2T_c = (adjN8_c @ s2q)^T / (N*256) + b2    [256, 2048] fp32

Big matmuls run in fp8-e4m3 DoubleRow (K=256/instr); small ones bf16.
Simulated end-to-end rel err ~1.55e-2 vs fp32 reference (tol 2e-2),
dominated by the fp8 quantization of x.
"""

import numpy as np
import ml_dtypes

import concourse.bass as bass
import concourse.mybir as mybir
import concourse.tile as tile
from concourse.bass_utils import run_bass_kernel_spmd
from concourse.tile_sem_assignment import N_PROCS
from concourse.vector_clock import ScopedClock, VectorClock

# ---------------------------------------------------------------------------
# Workaround: the walrus build in this container caps the number of sync-wait
# commands per instruction at ONE.  Tile's kernel-tail drain aggregates one
# wait per logical processor; split it into a chain of single-wait drains.
# Excess waits on regular instructions are hoisted onto no-ops.
# ---------------------------------------------------------------------------


def _drain_and_barrier_split(self, tick_clock, wait_clock):
    gc = tick_clock.global_clock
    for p in range(N_PROCS):
        partial = VectorClock([gc[q] if q == p else 0 for q in range(N_PROCS)])
        d = self.nc.sync.nop(nofuse=True)
        wait_clock.add_sem_waits(d.ins, ScopedClock({None: partial}))
    self.nc.sync.drain()

    self.nc.all_engine_barrier()
    assert self.sems is not None
    popped = self.nc._tile_sem_poison_stack.pop()
    assert popped is self._sem_poison
    self.nc.clear_and_free_semaphores(list(self.sems.allocated().values()))
    self.nc.all_engine_barrier()


tile.TileContext._drain_and_barrier = _drain_and_barrier_split

_MAX_WAITS = 1


def _split_excess_waits(nc):
    ctr = 0
    for f in nc.m.functions:
        for bb in f.blocks:
            out = []
            changed = False
            for inst in bb.instructions:
                si = inst.sync_info
                waits = list(si.on_wait) if si is not None and si.on_wait else []
                if len(waits) > _MAX_WAITS:
                    changed = True
                    keep, excess = waits[: _MAX_WAITS], waits[_MAX_WAITS :]
                    for i in range(0, len(excess), _MAX_WAITS):
                        ctr += 1
                        nop = mybir.InstNoOp(name=f"I-waitnop-{ctr}")
                        nop.engine = inst.engine
                        nop.sync_info = mybir.SyncInfo(
                            on_wait=excess[i : i + _MAX_WAITS], on_update=[]
                        )
                        out.append(nop)
                    si.on_wait = keep
                out.append(inst)
            if changed:
                bb.instructions = out
    return ctr


def _elide_redundant_ldweights(nc):
    """Drop an InstLdweights that reloads the same weights AP as the previous
    surviving one with only plain matmuls/no-ops in between (the PE keeps the
    stationary operand across matmuls; walrus emits one LDWEIGHTS per MATMUL)."""
    n_elided = 0
    for f in nc.m.functions:
        for bb in f.blocks:
            out = []
            last_w = None
            changed = False
            for inst in bb.instructions:
                nm = type(inst).__name__
                if nm == "InstLdweights":
                    si = inst.sync_info
                    clean = not (si and (si.on_wait or si.on_update))
                    w = repr(inst.ins[0])
                    if clean and last_w == w:
                        n_elided += 1
                        changed = True
                        continue
                    last_w = w if clean else None
                elif nm == "InstMatmult":
                    if getattr(inst, "is_transpose", False):
                        last_w = None
                elif nm == "InstNoOp":
                    pass
                else:
                    last_w = None
                out.append(inst)
            if changed:
                bb.instructions = out
    return n_elided


NCORES = 8
N = 16384
SH = N // NCORES  # 2048 adj rows per core
F = 512
HID = 1024
OUT = 256
S2SCALE = 256.0  # s2 is gathered as fp8 of 256*s2

BF16 = mybir.dt.bfloat16
F32 = mybir.dt.float32
FP8 = mybir.dt.float8e4
DR = mybir.MatmulPerfMode.DoubleRow

_built = None


def build():
    nc = bass.Bass()

    # adjU row r = kb*128 + p (k = kb*256 + kk*128 + p global col of adjT_c),
    # col = kk*2048 + i (i = local row of the adj shard), values N*adj in fp8.
    # P1 reads the 1024-wide i-half slices, P2 reads full rows.
    adjU = nc.declare_dram_parameter("adjU", [8192, 2 * SH], FP8, isOutput=False)
    # xP row = kb*128 + p, col = kk*512 + j
    xP = nc.declare_dram_parameter("xP", [8192, 2 * F], FP8, isOutput=False)
    w1n = nc.declare_dram_parameter("w1n", [F, HID], BF16, isOutput=False)  # W1/N
    w2 = nc.declare_dram_parameter("w2", [HID, OUT], BF16, isOutput=False)
    b1T = nc.declare_dram_parameter("b1T", [128, HID // 128], F32, isOutput=False)
    b2T = nc.declare_dram_parameter("b2T", [128, OUT // 128], F32, isOutput=False)
    out2T = nc.declare_dram_parameter("out2T", [OUT, SH], F32, isOutput=True)

    rg = [list(range(NCORES))]

    def allgather(inp, outp):
        return nc.gpsimd.collective_compute(
            "AllGather",
            mybir.AluOpType.bypass,
            replica_groups=rg,
            ins=[inp.opt()],
            outs=[outp.opt()],
        )

    with tile.TileContext(nc) as tc:
        with (
            tc.tile_pool(name="const", bufs=1) as constp,
            tc.tile_pool(name="psum", bufs=8, space="PSUM") as psum,
            tc.tile_pool(name="dram", bufs=1, space="DRAM") as dram,
            tc.tile_pool(name="adj", bufs=6) as adjp,
            tc.tile_pool(name="small", bufs=4) as smallp,
        ):
            # ---- constants / resident tensors ----
            # consts go on the ACT dma queue so the SP queue starts with the
            # x/adj tiles that gate the first matmul
            w1t = constp.tile([128, 4, HID], BF16)  # [j%128, jj, hid]
            nc.scalar.dma_start(w1t[:], w1n[:].rearrange("(jj p) h -> p jj h", p=128))
            w2t = constp.tile([128, 8, OUT], BF16)  # [hid%128, hh, j2]
            nc.scalar.dma_start(w2t[:], w2[:].rearrange("(hh p) o -> p hh o", p=128))
            b1t = constp.tile([128, 8], F32)
            nc.scalar.dma_start(b1t[:], b1T[:])
            b2t = constp.tile([128, 2], F32)
            nc.scalar.dma_start(b2t[:], b2T[:])
            # x streams per k-block (re-read in each half) -- cheaper than
            # keeping all 8MB resident; the freed SBUF holds more adj cache
            xP_r = xP[:].rearrange("(kb p) (kk j) -> p kb kk j", p=128, kk=2)

            # results kept in SBUF
            yT = constp.tile([128, 4, SH], BF16)  # [j%128, jj, i] = N*y
            hT = constp.tile([128, 8, SH], BF16)  # [hid%128, hh, i]

            # AllGather staging: chunk q covers local rows [512q, 512q+512)
            # laid out [p, kbl, kk, j2] (row = kbl*256 + kk*128 + p).
            ag_in = [dram.tile([128, 1024], FP8, name=f"agi{q}") for q in range(4)]
            ag_out = [
                dram.tile([NCORES * 128, 1024], FP8, addr_space="Shared", name=f"ago{q}")
                for q in range(4)
            ]

            adjU_r = adjU[:].rearrange("(kb p) (kk i) -> p kb kk i", p=128, kk=2)

            # P2 is DMA-bandwidth-bound: keep adj k-blocks loaded during P1
            # resident in SBUF so P2 skips their reload (saves 10MB of the
            # ~36MB P2 stream).  Spread across the AG-arrival groups.
            CACHE_KBS = tuple(range(0, 40, 2))
            adj_cache = {}

            for H in range(2):
                # ---- P1 half H: psY[j, i-1024-half] += x8^T adjC8 ----
                psY = [
                    psum.tile([128, 512], F32, tag="ps", name=f"psY{H}{t}")
                    for t in range(8)
                ]  # tile t = (jj, b): jj*2 + b; holds i-chunks 2b, 2b+1
                for kb in range(64):
                    xt = smallp.tile([128, 2, F], FP8, tag="xt", bufs=8)
                    nc.sync.dma_start(xt[:], xP_r[:, kb])
                    if kb in CACHE_KBS:
                        at = constp.tile([128, 2, 1024], FP8, name=f"ac_{H}_{kb}")
                        adj_cache[(H, kb)] = at
                    else:
                        at = adjp.tile(
                            [128, 2, 1024], FP8, tag="adjt", name=f"a1_{H}_{kb}"
                        )
                    nc.sync.dma_start(
                        at[:], adjU_r[:, kb, :, H * 1024 : (H + 1) * 1024]
                    )
                    for jj in range(4):
                        lhs = xt[:, :, jj * 128 : (jj + 1) * 128]
                        for c in range(4):  # i-chunk of 256 within the half
                            nc.tensor.matmul(
                                psY[jj * 2 + c // 2][:, (c % 2) * 256 : (c % 2) * 256 + 256],
                                lhs,
                                at[:, :, c * 256 : (c + 1) * 256],
                                start=(kb == 0 and c % 2 == 0),
                                stop=(kb == 63 and c % 2 == 1),
                                perf_mode=DR,
                            )
                # drain psY -> yT (bf16) on the vector engine (ACT is busier)
                for jj in range(4):
                    for b in range(2):
                        nc.vector.tensor_copy(
                            yT[:, jj, H * 1024 + b * 512 : H * 1024 + b * 512 + 512],
                            psY[jj * 2 + b][:],
                        )

                # ---- supportT + relu: hT = relu(W1n^T yT + b1) ----
                # hh-groups of 4 with i-width 1024: each stationary W1 block
                # feeds two 512-wide matmuls, halving LDWEIGHTS count.
                for hg in range(2):
                    i0 = H * 1024
                    psS = [
                        psum.tile([128, 512], F32, tag="ps", name=f"psS{H}{hg}{t}")
                        for t in range(8)
                    ]  # t = hh4*2 + qq
                    for hh4 in range(4):
                        hh = hg * 4 + hh4
                        for jj in range(4):
                            for qq in range(2):
                                nc.tensor.matmul(
                                    psS[hh4 * 2 + qq][:],
                                    w1t[:, jj, hh * 128 : (hh + 1) * 128],
                                    yT[:, jj, i0 + qq * 512 : i0 + qq * 512 + 512],
                                    start=(jj == 0),
                                    stop=(jj == 3),
                                )
                    # relu drains split ACT/DVE: the first s2 matmul group is
                    # paced by these, so halve the chain latency
                    for hh4 in range(4):
                        hh = hg * 4 + hh4
                        for qq in range(2):
                            dst = hT[:, hh, i0 + qq * 512 : i0 + qq * 512 + 512]
                            src = psS[hh4 * 2 + qq][:]
                            if qq == 1:
                                nc.vector.tensor_scalar(
                                    dst,
                                    src,
                                    b1t[:, hh : hh + 1],
                                    0.0,
                                    op0=mybir.AluOpType.add,
                                    op1=mybir.AluOpType.max,
                                )
                            else:
                                nc.scalar.activation(
                                    dst,
                                    src,
                                    mybir.ActivationFunctionType.Relu,
                                    bias=b1t[:, hh : hh + 1],
                                )

                # ---- s2 = h @ W2, quantized fp8*256, staged for AG ----
                # psum/AG column order (j2t, kk, jp) so P2's stationary load
                # is a plain contiguous copy.
                for qq in range(2):
                    q = H * 2 + qq  # global chunk id
                    for kbl in range(2):
                        ps2 = psum.tile(
                            [128, 2, 2, 128], F32, tag="ps", name=f"ps2{q}{kbl}"
                        )
                        for kk in range(2):
                            i0 = q * 512 + kbl * 256 + kk * 128
                            for hh in range(8):
                                nc.tensor.matmul(
                                    ps2[:, :, kk, :],
                                    hT[:, hh, i0 : i0 + 128],
                                    w2t[:, hh, :],
                                    start=(hh == 0 and kk == 0),
                                    stop=(hh == 7 and kk == 1),
                                )
                        # drains alternate ACT/DVE and stores go on gpsimd:
                        # these COPYs release the psum banks the next phase's
                        # matmuls reuse, so their chain latency is exposed
                        s2q = smallp.tile([128, 512], FP8, tag="s2q", bufs=4)
                        if kbl == 0:
                            nc.scalar.activation(
                                s2q[:],
                                ps2[:].rearrange("p a b j -> p (a b j)"),
                                mybir.ActivationFunctionType.Copy,
                                scale=S2SCALE,
                            )
                        else:
                            nc.vector.tensor_scalar(
                                s2q[:],
                                ps2[:].rearrange("p a b j -> p (a b j)"),
                                S2SCALE,
                                None,
                                op0=mybir.AluOpType.mult,
                            )
                        nc.gpsimd.dma_start(
                            ag_in[q][:, kbl * 512 : kbl * 512 + 512], s2q[:]
                        )
                    allgather(ag_in[q], ag_out[q])

            # ---- P2: out2T = (adjU8 @ s2q)^T / (N*256) + b2 ----
            psD = [
                psum.tile([128, 512], F32, tag="ps", name=f"psD{t}") for t in range(8)
            ]  # tile t = (j2, cb): j2*4 + cb; holds i-chunks 2cb, 2cb+1
            kb_order = [
                8 * c + 2 * q + t for q in range(4) for c in range(NCORES) for t in range(2)
            ]
            for ki, kb in enumerate(kb_order):
                c, rem = kb // 8, kb % 8
                q, t = rem // 2, rem % 2
                if kb in CACHE_KBS:
                    at = None  # rhs comes from the SBUF-cached P1 tiles
                else:
                    at = adjp.tile([128, 2, 2048], FP8, tag="adjt", name=f"a2_{kb}")
                    nc.sync.dma_start(at[:], adjU_r[:, kb])
                st = smallp.tile([128, 2, 2, 128], FP8, tag="st", bufs=4, name=f"st{kb}")
                # ag_out rows c*128+p, cols t*512 + j2t*256 + kk*128 + jp
                nc.sync.dma_start(
                    st[:].rearrange("p a b j -> p (a b j)"),
                    ag_out[q][c * 128 : (c + 1) * 128, t * 512 : (t + 1) * 512],
                )
                for j2 in range(2):
                    lhs = st[:, j2]
                    for c8 in range(8):
                        if at is not None:
                            rhs = at[:, :, c8 * 256 : (c8 + 1) * 256]
                        else:
                            cc = c8 % 4
                            rhs = adj_cache[(c8 // 4, kb)][
                                :, :, cc * 256 : (cc + 1) * 256
                            ]
                        nc.tensor.matmul(
                            psD[j2 * 4 + c8 // 2][:, (c8 % 2) * 256 : (c8 % 2) * 256 + 256],
                            lhs,
                            rhs,
                            start=(ki == 0 and c8 % 2 == 0),
                            stop=(ki == 63 and c8 % 2 == 1),
                            perf_mode=DR,
                        )
            # final drain split across ACT and DVE so the tail is ~2x shorter
            for j2 in range(2):
                for cb in range(4):
                    ot = smallp.tile([128, 512], F32, tag="ot", bufs=4)
                    if cb % 2 == 0:
                        nc.scalar.activation(
                            ot[:],
                            psD[j2 * 4 + cb][:],
                            mybir.ActivationFunctionType.Identity,
                            bias=b2t[:, j2 : j2 + 1],
                            scale=1.0 / (N * S2SCALE),
                        )
                        nc.scalar.dma_start(
                            out2T[j2 * 128 : (j2 + 1) * 128, cb * 512 : (cb + 1) * 512],
                            ot[:],
                        )
                    else:
                        nc.vector.tensor_scalar(
                            ot[:],
                            psD[j2 * 4 + cb][:],
                            1.0 / (N * S2SCALE),
                            b2t[:, j2 : j2 + 1],
                            op0=mybir.AluOpType.mult,
                            op1=mybir.AluOpType.add,
                        )
                        nc.gpsimd.dma_start(
                            out2T[j2 * 128 : (j2 + 1) * 128, cb * 512 : (cb + 1) * 512],
                            ot[:],
                        )

    _elide_redundant_ldweights(nc)
    _split_excess_waits(nc)
    return nc


def _prep_inputs(x, adj, W1, b1, W2, b2):
    bf = ml_dtypes.bfloat16
    f8 = ml_dtypes.float8_e4m3fn

    u = adj * np.float32(N)  # exact: adj was u/N with N a power of two
    u8 = u.astype(f8)
    x8 = x.astype(f8)
    b1T = np.ascontiguousarray(b1.reshape(HID // 128, 128).T).astype(np.float32)
    b2T = np.ascontiguousarray(b2.reshape(OUT // 128, 128).T).astype(np.float32)
    w1n = (W1 / np.float32(N)).astype(bf)
    w2b = W2.astype(bf)
    # xP[kb*128+p, kk*512+j] = x8[kb*256+kk*128+p, j]
    xP = np.ascontiguousarray(
        x8.reshape(64, 2, 128, F).transpose(0, 2, 1, 3).reshape(8192, 2 * F)
    )

    def adj_layout(a8, rows):
        # out[kb*128+p, kk*2048+i] = a8[rows][i, kb*256+kk*128+p]
        blk = a8[rows, :].reshape(SH, 64, 2, 128)  # [i, kb, kk, p]
        return np.ascontiguousarray(
            blk.transpose(1, 3, 2, 0).reshape(8192, 2 * SH)
        )

    in_maps = []
    for c in range(NCORES):
        rows = slice(c * SH, (c + 1) * SH)
        in_maps.append(
            {
                "adjU": adj_layout(u8, rows),
                "xP": xP,
                "w1n": w1n,
                "w2": w2b,
                "b1T": b1T,
                "b2T": b2T,
            }
        )
    return in_maps


def _run(inputs, trace=False):
    global _built
    if _built is None:
        _built = build()
    in_maps = _prep_inputs(**inputs)
    r = run_bass_kernel_spmd(_built, in_maps, list(range(NCORES)), trace=trace)
    out = np.empty([N, OUT], np.float32)
    for c in range(NCORES):
        out[c * SH : (c + 1) * SH, :] = r.results[c]["out2T"].T
    return out, r


def kernel(x, adj, W1, b1, W2, b2):
    out, _ = _run(dict(x=x, adj=adj, W1=W1, b1=b1, W2=W2, b2=b2))
    return out


# revision 59
# speedup vs baseline: 1.0195x; 1.0075x over previous
"""Trainium2 Bass kernel for a 2-layer dense GCN (NodeEncoder).

    out = adj @ relu(adj @ (x@W1) + b1) @ W2 + b2
    N=16384, F_IN=512, HID=1024, OUT=256, adj dense [N, N] fp32.

Algorithm (reassociated to nearly halve layer-1 FLOPs and drop the big
s1 AllGather):  relu(adj @ (x@W1)) == relu((adj@x) @ W1), so per core
(adj row-partitioned, 2048 rows each):

  P1:    yT_c   = x8^T @ adjN8_c^T                     [512, 2048]  (= N*y^T)
  small: hT_c   = relu(yT_c^T @ (W1/N) + b1)^T         [1024, 2048] bf16
         s2_c   = h_c @ W2                             [2048, 256]
         quantized to fp8 * 256 for the gather.
  AG:    s2q    = AllGather(s2q_c)  (4 chunks of 128KB, overlapped)
  P2:    out2T_c = (adjN8_c @ s2q)^T / (N*256) + b2    [256, 2048] fp32

Big matmuls run in fp8-e4m3 DoubleRow (K=256/instr); small ones bf16.
Simulated end-to-end rel err ~1.55e-2 vs fp32 reference (tol 2e-2),
dominated by the fp8 quantization of x.
"""

import numpy as np
import ml_dtypes

import concourse.bass as bass
import concourse.mybir as mybir
import concourse.tile as tile
from concourse.bass_utils import run_bass_kernel_spmd
from concourse.tile_sem_assignment import N_PROCS
from concourse.vector_clock import ScopedClock, VectorClock

# ---------------------------------------------------------------------------
# Workaround: the walrus build in this container caps the number of sync-wait
# commands per instruction at ONE.  Tile's kernel-tail drain aggregates one
# wait per logical processor; split it into a chain of single-wait drains.
# Excess waits on regular instructions are hoisted onto no-ops.
# ---------------------------------------------------------------------------


def _drain_and_barrier_split(self, tick_clock, wait_clock):
    gc = tick_clock.global_clock
    for p in range(N_PROCS):
        partial = VectorClock([gc[q] if q == p else 0 for q in range(N_PROCS)])
        d = self.nc.sync.nop(nofuse=True)
        wait_clock.add_sem_waits(d.ins, ScopedClock({None: partial}))
    self.nc.sync.drain()

    self.nc.all_engine_barrier()
    assert self.sems is not None
    popped = self.nc._tile_sem_poison_stack.pop()
    assert popped is self._sem_poison
    self.nc.clear_and_free_semaphores(list(self.sems.allocated().values()))
    self.nc.all_engine_barrier()


tile.TileContext._drain_and_barrier = _drain_and_barrier_split

_MAX_WAITS = 1


def _split_excess_waits(nc):
    ctr = 0
    for f in nc.m.functions:
        for bb in f.blocks:
            out = []
            changed = False
            for inst in bb.instructions:
                si = inst.sync_info
                waits = list(si.on_wait) if si is not None and si.on_wait else []
                if len(waits) > _MAX_WAITS:
                    changed = True
                    keep, excess = waits[: _MAX_WAITS], waits[_MAX_WAITS :]
                    for i in range(0, len(excess), _MAX_WAITS):
                        ctr += 1
                        nop = mybir.InstNoOp(name=f"I-waitnop-{ctr}")
                        nop.engine = inst.engine
                        nop.sync_info = mybir.SyncInfo(
                            on_wait=excess[i : i + _MAX_WAITS], on_update=[]
                        )
                        out.append(nop)
                    si.on_wait = keep
                out.append(inst)
            if changed:
                bb.instructions = out
    return ctr


def _elide_redundant_ldweights(nc):
    """Drop an InstLdweights that reloads the same weights AP as the previous
    surviving one with only plain matmuls/no-ops in between (the PE keeps the
    stationary operand across matmuls; walrus emits one LDWEIGHTS per MATMUL)."""
    n_elided = 0
    for f in nc.m.functions:
        for bb in f.blocks:
            out = []
            last_w = None
            changed = False
            for inst in bb.instructions:
                nm = type(inst).__name__
                if nm == "InstLdweights":
                    si = inst.sync_info
                    clean = not (si and (si.on_wait or si.on_update))
                    w = repr(inst.ins[0])
                    if clean and last_w == w:
                        n_elided += 1
                        changed = True
                        continue
                    last_w = w if clean else None
                elif nm == "InstMatmult":
                    if getattr(inst, "is_transpose", False):
                        last_w = None
                elif nm == "InstNoOp":
                    pass
                else:
                    last_w = None
                out.append(inst)
            if changed:
                bb.instructions = out
    return n_elided


NCORES = 8
N = 16384
SH = N // NCORES  # 2048 adj rows per core
F = 512
HID = 1024
OUT = 256
S2SCALE = 256.0  # s2 is gathered as fp8 of 256*s2

BF16 = mybir.dt.bfloat16
F16 = mybir.dt.float16
F32 = mybir.dt.float32
FP8 = mybir.dt.float8e4
DR = mybir.MatmulPerfMode.DoubleRow

_built = None


def build():
    nc = bass.Bass()

    # adjU row r = kb*128 + p (k = kb*256 + kk*128 + p global col of adjT_c),
    # col = kk*2048 + i (i = local row of the adj shard), values N*adj in fp8.
    # P1 reads the 1024-wide i-half slices, P2 reads full rows.
    adjU = nc.declare_dram_parameter("adjU", [8192, 2 * SH], FP8, isOutput=False)
    # xP row = kb*128 + p, col = kk*512 + j
    xP = nc.declare_dram_parameter("xP", [8192, 2 * F], FP8, isOutput=False)
    w1n = nc.declare_dram_parameter("w1n", [F, HID], BF16, isOutput=False)  # W1/N
    w2 = nc.declare_dram_parameter("w2", [HID, OUT], BF16, isOutput=False)
    b1T = nc.declare_dram_parameter("b1T", [128, HID // 128], F32, isOutput=False)
    b2T = nc.declare_dram_parameter("b2T", [128, OUT // 128], F32, isOutput=False)
    out2T = nc.declare_dram_parameter("out2T", [OUT, SH], F16, isOutput=True)

    rg = [list(range(NCORES))]

    def allgather(inp, outp):
        return nc.gpsimd.collective_compute(
            "AllGather",
            mybir.AluOpType.bypass,
            replica_groups=rg,
            ins=[inp.opt()],
            outs=[outp.opt()],
        )

    with tile.TileContext(nc) as tc:
        with (
            tc.tile_pool(name="const", bufs=1) as constp,
            tc.tile_pool(name="psum", bufs=8, space="PSUM") as psum,
            tc.tile_pool(name="dram", bufs=1, space="DRAM") as dram,
            tc.tile_pool(name="adj", bufs=6) as adjp,
            tc.tile_pool(name="small", bufs=4) as smallp,
        ):
            # ---- constants / resident tensors ----
            # consts go on the ACT dma queue so the SP queue starts with the
            # x/adj tiles that gate the first matmul
            w1t = constp.tile([128, 4, HID], BF16)  # [j%128, jj, hid]
            nc.scalar.dma_start(w1t[:], w1n[:].rearrange("(jj p) h -> p jj h", p=128))
            w2t = constp.tile([128, 8, OUT], BF16)  # [hid%128, hh, j2]
            nc.scalar.dma_start(w2t[:], w2[:].rearrange("(hh p) o -> p hh o", p=128))
            b1t = constp.tile([128, 8], F32)
            nc.scalar.dma_start(b1t[:], b1T[:])
            b2t = constp.tile([128, 2], F32)
            nc.scalar.dma_start(b2t[:], b2T[:])
            # x streams per k-block (re-read in each half) -- cheaper than
            # keeping all 8MB resident; the freed SBUF holds more adj cache
            xP_r = xP[:].rearrange("(kb p) (kk j) -> p kb kk j", p=128, kk=2)

            # results kept in SBUF
            yT = constp.tile([128, 4, SH], BF16)  # [j%128, jj, i] = N*y
            hT = constp.tile([128, 8, SH], BF16)  # [hid%128, hh, i]

            # AllGather staging: chunk q covers local rows [512q, 512q+512)
            # laid out [p, kbl, kk, j2] (row = kbl*256 + kk*128 + p).
            ag_in = [dram.tile([128, 1024], FP8, name=f"agi{q}") for q in range(4)]
            ag_out = [
                dram.tile([NCORES * 128, 1024], FP8, addr_space="Shared", name=f"ago{q}")
                for q in range(4)
            ]

            adjU_r = adjU[:].rearrange("(kb p) (kk i) -> p kb kk i", p=128, kk=2)

            # P2 is DMA-bandwidth-bound: keep adj k-blocks loaded during P1
            # resident in SBUF so P2 skips their reload (saves 10MB of the
            # ~36MB P2 stream).  Spread across the AG-arrival groups.
            CACHE_KBS = tuple(range(0, 40, 2))
            adj_cache = {}

            for H in range(2):
                # ---- P1 half H: psY[j, i-1024-half] += x8^T adjC8 ----
                psY = [
                    psum.tile([128, 512], F32, tag="ps", name=f"psY{H}{t}")
                    for t in range(8)
                ]  # tile t = (jj, b): jj*2 + b; holds i-chunks 2b, 2b+1
                for kb in range(64):
                    xt = smallp.tile([128, 2, F], FP8, tag="xt", bufs=8)
                    nc.sync.dma_start(xt[:], xP_r[:, kb])
                    if kb in CACHE_KBS:
                        at = constp.tile([128, 2, 1024], FP8, name=f"ac_{H}_{kb}")
                        adj_cache[(H, kb)] = at
                    else:
                        at = adjp.tile(
                            [128, 2, 1024], FP8, tag="adjt", name=f"a1_{H}_{kb}"
                        )
                    nc.sync.dma_start(
                        at[:], adjU_r[:, kb, :, H * 1024 : (H + 1) * 1024]
                    )
                    for jj in range(4):
                        lhs = xt[:, :, jj * 128 : (jj + 1) * 128]
                        for c in range(4):  # i-chunk of 256 within the half
                            nc.tensor.matmul(
                                psY[jj * 2 + c // 2][:, (c % 2) * 256 : (c % 2) * 256 + 256],
                                lhs,
                                at[:, :, c * 256 : (c + 1) * 256],
                                start=(kb == 0 and c % 2 == 0),
                                stop=(kb == 63 and c % 2 == 1),
                                perf_mode=DR,
                            )
                # drain psY -> yT (bf16) on the vector engine (ACT is busier)
                for jj in range(4):
                    for b in range(2):
                        nc.vector.tensor_copy(
                            yT[:, jj, H * 1024 + b * 512 : H * 1024 + b * 512 + 512],
                            psY[jj * 2 + b][:],
                        )

                # ---- supportT + relu: hT = relu(W1n^T yT + b1) ----
                # hh-groups of 4 with i-width 1024: each stationary W1 block
                # feeds two 512-wide matmuls, halving LDWEIGHTS count.
                for hg in range(2):
                    i0 = H * 1024
                    psS = [
                        psum.tile([128, 512], F32, tag="ps", name=f"psS{H}{hg}{t}")
                        for t in range(8)
                    ]  # t = hh4*2 + qq
                    for hh4 in range(4):
                        hh = hg * 4 + hh4
                        for jj in range(4):
                            for qq in range(2):
                                nc.tensor.matmul(
                                    psS[hh4 * 2 + qq][:],
                                    w1t[:, jj, hh * 128 : (hh + 1) * 128],
                                    yT[:, jj, i0 + qq * 512 : i0 + qq * 512 + 512],
                                    start=(jj == 0),
                                    stop=(jj == 3),
                                )
                    # relu drains split ACT/DVE: the first s2 matmul group is
                    # paced by these, so halve the chain latency
                    for hh4 in range(4):
                        hh = hg * 4 + hh4
                        for qq in range(2):
                            dst = hT[:, hh, i0 + qq * 512 : i0 + qq * 512 + 512]
                            src = psS[hh4 * 2 + qq][:]
                            if qq == 1:
                                nc.vector.tensor_scalar(
                                    dst,
                                    src,
                                    b1t[:, hh : hh + 1],
                                    0.0,
                                    op0=mybir.AluOpType.add,
                                    op1=mybir.AluOpType.max,
                                )
                            else:
                                nc.scalar.activation(
                                    dst,
                                    src,
                                    mybir.ActivationFunctionType.Relu,
                                    bias=b1t[:, hh : hh + 1],
                                )

                # ---- s2 = h @ W2, quantized fp8*256, staged for AG ----
                # psum/AG column order (j2t, kk, jp) so P2's stationary load
                # is a plain contiguous copy.
                for qq in range(2):
                    q = H * 2 + qq  # global chunk id
                    for kbl in range(2):
                        ps2 = psum.tile(
                            [128, 2, 2, 128], F32, tag="ps", name=f"ps2{q}{kbl}"
                        )
                        for kk in range(2):
                            i0 = q * 512 + kbl * 256 + kk * 128
                            for hh in range(8):
                                nc.tensor.matmul(
                                    ps2[:, :, kk, :],
                                    hT[:, hh, i0 : i0 + 128],
                                    w2t[:, hh, :],
                                    start=(hh == 0 and kk == 0),
                                    stop=(hh == 7 and kk == 1),
                                )
                        # drains alternate ACT/DVE and stores go on gpsimd:
                        # these COPYs release the psum banks the next phase's
                        # matmuls reuse, so their chain latency is exposed
                        s2q = smallp.tile([128, 512], FP8, tag="s2q", bufs=4)
                        if kbl == 0:
                            nc.scalar.activation(
                                s2q[:],
                                ps2[:].rearrange("p a b j -> p (a b j)"),
                                mybir.ActivationFunctionType.Copy,
                                scale=S2SCALE,
                            )
                        else:
                            nc.vector.tensor_scalar(
                                s2q[:],
                                ps2[:].rearrange("p a b j -> p (a b j)"),
                                S2SCALE,
                                None,
                                op0=mybir.AluOpType.mult,
                            )
                        nc.gpsimd.dma_start(
                            ag_in[q][:, kbl * 512 : kbl * 512 + 512], s2q[:]
                        )
                    allgather(ag_in[q], ag_out[q])

            # ---- P2: out2T = (adjU8 @ s2q)^T / (N*256) + b2 ----
            psD = [
                psum.tile([128, 512], F32, tag="ps", name=f"psD{t}") for t in range(8)
            ]  # tile t = (j2, cb): j2*4 + cb; holds i-chunks 2cb, 2cb+1
            kb_order = [
                8 * c + 2 * q + t for q in range(4) for c in range(NCORES) for t in range(2)
            ]
            for ki, kb in enumerate(kb_order):
                c, rem = kb // 8, kb % 8
                q, t = rem // 2, rem % 2
                if kb in CACHE_KBS:
                    at = None  # rhs comes from the SBUF-cached P1 tiles
                else:
                    at = adjp.tile([128, 2, 2048], FP8, tag="adjt", name=f"a2_{kb}")
                    nc.sync.dma_start(at[:], adjU_r[:, kb])
                st = smallp.tile([128, 2, 2, 128], FP8, tag="st", bufs=4, name=f"st{kb}")
                # ag_out rows c*128+p, cols t*512 + j2t*256 + kk*128 + jp
                nc.sync.dma_start(
                    st[:].rearrange("p a b j -> p (a b j)"),
                    ag_out[q][c * 128 : (c + 1) * 128, t * 512 : (t + 1) * 512],
                )
                for j2 in range(2):
                    lhs = st[:, j2]
                    for c8 in range(8):
                        if at is not None:
                            rhs = at[:, :, c8 * 256 : (c8 + 1) * 256]
                        else:
                            cc = c8 % 4
                            rhs = adj_cache[(c8 // 4, kb)][
                                :, :, cc * 256 : (cc + 1) * 256
                            ]
                        nc.tensor.matmul(
                            psD[j2 * 4 + c8 // 2][:, (c8 % 2) * 256 : (c8 % 2) * 256 + 256],
                            lhs,
                            rhs,
                            start=(ki == 0 and c8 % 2 == 0),
                            stop=(ki == 63 and c8 % 2 == 1),
                            perf_mode=DR,
                        )
            # final drain split across ACT and DVE so the tail is ~2x shorter
            for j2 in range(2):
                for cb in range(4):
                    ot = smallp.tile([128, 512], F16, tag="ot", bufs=4)
                    if cb % 2 == 0:
                        nc.scalar.activation(
                            ot[:],
                            psD[j2 * 4 + cb][:],
                            mybir.ActivationFunctionType.Identity,
                            bias=b2t[:, j2 : j2 + 1],
                            scale=1.0 / (N * S2SCALE),
                        )
                        nc.scalar.dma_start(
                            out2T[j2 * 128 : (j2 + 1) * 128, cb * 512 : (cb + 1) * 512],
                            ot[:],
                        )
                    else:
                        nc.vector.tensor_scalar(
                            ot[:],
                            psD[j2 * 4 + cb][:],
                            1.0 / (N * S2SCALE),
                            b2t[:, j2 : j2 + 1],
                            op0=mybir.AluOpType.mult,
                            op1=mybir.AluOpType.add,
                        )
                        nc.gpsimd.dma_start(
                            out2T[j2 * 128 : (j2 + 1) * 128, cb * 512 : (cb + 1) * 512],
                            ot[:],
                        )

    _elide_redundant_ldweights(nc)
    _split_excess_waits(nc)
    return nc


def _prep_inputs(x, adj, W1, b1, W2, b2):
    bf = ml_dtypes.bfloat16
    f8 = ml_dtypes.float8_e4m3fn

    u = adj * np.float32(N)  # exact: adj was u/N with N a power of two
    u8 = u.astype(f8)
    x8 = x.astype(f8)
    b1T = np.ascontiguousarray(b1.reshape(HID // 128, 128).T).astype(np.float32)
    b2T = np.ascontiguousarray(b2.reshape(OUT // 128, 128).T).astype(np.float32)
    w1n = (W1 / np.float32(N)).astype(bf)
    w2b = W2.astype(bf)
    # xP[kb*128+p, kk*512+j] = x8[kb*256+kk*128+p, j]
    xP = np.ascontiguousarray(
        x8.reshape(64, 2, 128, F).transpose(0, 2, 1, 3).reshape(8192, 2 * F)
    )

    def adj_layout(a8, rows):
        # out[kb*128+p, kk*2048+i] = a8[rows][i, kb*256+kk*128+p]
        blk = a8[rows, :].reshape(SH, 64, 2, 128)  # [i, kb, kk, p]
        return np.ascontiguousarray(
            blk.transpose(1, 3, 2, 0).reshape(8192, 2 * SH)
        )

    in_maps = []
    for c in range(NCORES):
        rows = slice(c * SH, (c + 1) * SH)
        in_maps.append(
            {
                "adjU": adj_layout(u8, rows),
                "xP": xP,
                "w1n": w1n,
                "w2": w2b,
                "b1T": b1T,
                "b2T": b2T,
            }
        )
    return in_maps


def _run(inputs, trace=False):
    global _built
    if _built is None:
        _built = build()
    in_maps = _prep_inputs(**inputs)
    r = run_bass_kernel_spmd(_built, in_maps, list(range(NCORES)), trace=trace)
    out = np.empty([N, OUT], np.float32)
    for c in range(NCORES):
        out[c * SH : (c + 1) * SH, :] = r.results[c]["out2T"].T.astype(np.float32)
    return out, r


def kernel(x, adj, W1, b1, W2, b2):
    out, _ = _run(dict(x=x, adj=adj, W1=W1, b1=b1, W2=W2, b2=b2))
    return out


# revision 60
# speedup vs baseline: 1.0195x; 1.0000x over previous
"""Trainium2 Bass kernel for a 2-layer dense GCN (NodeEncoder).

    out = adj @ relu(adj @ (x@W1) + b1) @ W2 + b2
    N=16384, F_IN=512, HID=1024, OUT=256, adj dense [N, N] fp32.

Algorithm (reassociated to nearly halve layer-1 FLOPs and drop the big
s1 AllGather):  relu(adj @ (x@W1)) == relu((adj@x) @ W1), so per core
(adj row-partitioned, 2048 rows each):

  P1:    yT_c   = x8^T @ adjN8_c^T                     [512, 2048]  (= N*y^T)
  small: hT_c   = relu(yT_c^T @ (W1/N) + b1)^T         [1024, 2048] bf16
         s2_c   = h_c @ W2                             [2048, 256]
         quantized to fp8 * 256 for the gather.
  AG:    s2q    = AllGather(s2q_c)  (4 chunks of 128KB, overlapped)
  P2:    out2T_c = (adjN8_c @ s2q)^T / (N*256) + b2    [256, 2048] fp32

Big matmuls run in fp8-e4m3 DoubleRow (K=256/instr); small ones bf16.
Simulated end-to-end rel err ~1.55e-2 vs fp32 reference (tol 2e-2),
dominated by the fp8 quantization of x.
"""

import numpy as np
import ml_dtypes

import concourse.bass as bass
import concourse.mybir as mybir
import concourse.tile as tile
from concourse.bass_utils import run_bass_kernel_spmd
from concourse.tile_sem_assignment import N_PROCS
from concourse.vector_clock import ScopedClock, VectorClock

# ---------------------------------------------------------------------------
# Workaround: the walrus build in this container caps the number of sync-wait
# commands per instruction at ONE.  Tile's kernel-tail drain aggregates one
# wait per logical processor; split it into a chain of single-wait drains.
# Excess waits on regular instructions are hoisted onto no-ops.
# ---------------------------------------------------------------------------


def _drain_and_barrier_split(self, tick_clock, wait_clock):
    gc = tick_clock.global_clock
    for p in range(N_PROCS):
        partial = VectorClock([gc[q] if q == p else 0 for q in range(N_PROCS)])
        d = self.nc.sync.nop(nofuse=True)
        wait_clock.add_sem_waits(d.ins, ScopedClock({None: partial}))
    self.nc.sync.drain()

    self.nc.all_engine_barrier()
    assert self.sems is not None
    popped = self.nc._tile_sem_poison_stack.pop()
    assert popped is self._sem_poison
    self.nc.clear_and_free_semaphores(list(self.sems.allocated().values()))
    self.nc.all_engine_barrier()


tile.TileContext._drain_and_barrier = _drain_and_barrier_split

_MAX_WAITS = 1


def _split_excess_waits(nc):
    ctr = 0
    for f in nc.m.functions:
        for bb in f.blocks:
            out = []
            changed = False
            for inst in bb.instructions:
                si = inst.sync_info
                waits = list(si.on_wait) if si is not None and si.on_wait else []
                if len(waits) > _MAX_WAITS:
                    changed = True
                    keep, excess = waits[: _MAX_WAITS], waits[_MAX_WAITS :]
                    for i in range(0, len(excess), _MAX_WAITS):
                        ctr += 1
                        nop = mybir.InstNoOp(name=f"I-waitnop-{ctr}")
                        nop.engine = inst.engine
                        nop.sync_info = mybir.SyncInfo(
                            on_wait=excess[i : i + _MAX_WAITS], on_update=[]
                        )
                        out.append(nop)
                    si.on_wait = keep
                out.append(inst)
            if changed:
                bb.instructions = out
    return ctr


def _elide_redundant_ldweights(nc):
    """Drop an InstLdweights that reloads the same weights AP as the previous
    surviving one with only plain matmuls/no-ops in between (the PE keeps the
    stationary operand across matmuls; walrus emits one LDWEIGHTS per MATMUL)."""
    n_elided = 0
    for f in nc.m.functions:
        for bb in f.blocks:
            out = []
            last_w = None
            changed = False
            for inst in bb.instructions:
                nm = type(inst).__name__
                if nm == "InstLdweights":
                    si = inst.sync_info
                    clean = not (si and (si.on_wait or si.on_update))
                    w = repr(inst.ins[0])
                    if clean and last_w == w:
                        n_elided += 1
                        changed = True
                        continue
                    last_w = w if clean else None
                elif nm == "InstMatmult":
                    if getattr(inst, "is_transpose", False):
                        last_w = None
                elif nm == "InstNoOp":
                    pass
                else:
                    last_w = None
                out.append(inst)
            if changed:
                bb.instructions = out
    return n_elided


NCORES = 8
N = 16384
SH = N // NCORES  # 2048 adj rows per core
F = 512
HID = 1024
OUT = 256
S2SCALE = 256.0  # s2 is gathered as fp8 of 256*s2

BF16 = mybir.dt.bfloat16
F16 = mybir.dt.float16
F32 = mybir.dt.float32
FP8 = mybir.dt.float8e4
DR = mybir.MatmulPerfMode.DoubleRow

_built = None


def build():
    nc = bass.Bass()

    # adjU row r = kb*128 + p (k = kb*256 + kk*128 + p global col of adjT_c),
    # col = kk*2048 + i (i = local row of the adj shard), values N*adj in fp8.
    # P1 reads the 1024-wide i-half slices, P2 reads full rows.
    adjU = nc.declare_dram_parameter("adjU", [8192, 2 * SH], FP8, isOutput=False)
    # xP row = kb*128 + p, col = kk*512 + j
    xP = nc.declare_dram_parameter("xP", [8192, 2 * F], FP8, isOutput=False)
    w1n = nc.declare_dram_parameter("w1n", [F, HID], BF16, isOutput=False)  # W1/N
    w2 = nc.declare_dram_parameter("w2", [HID, OUT], BF16, isOutput=False)
    b1T = nc.declare_dram_parameter("b1T", [128, HID // 128], F32, isOutput=False)
    b2T = nc.declare_dram_parameter("b2T", [128, OUT // 128], F32, isOutput=False)
    out2T = nc.declare_dram_parameter("out2T", [OUT, SH], F16, isOutput=True)

    rg = [list(range(NCORES))]

    def allgather(inp, outp):
        return nc.gpsimd.collective_compute(
            "AllGather",
            mybir.AluOpType.bypass,
            replica_groups=rg,
            ins=[inp.opt()],
            outs=[outp.opt()],
        )

    with tile.TileContext(nc) as tc:
        with (
            tc.tile_pool(name="const", bufs=1) as constp,
            tc.tile_pool(name="psum", bufs=8, space="PSUM") as psum,
            tc.tile_pool(name="dram", bufs=1, space="DRAM") as dram,
            tc.tile_pool(name="adj", bufs=6) as adjp,
            tc.tile_pool(name="small", bufs=4) as smallp,
        ):
            # ---- constants / resident tensors ----
            # consts go on the ACT dma queue so the SP queue starts with the
            # x/adj tiles that gate the first matmul
            w1t = constp.tile([128, 4, HID], BF16)  # [j%128, jj, hid]
            nc.scalar.dma_start(w1t[:], w1n[:].rearrange("(jj p) h -> p jj h", p=128))
            w2t = constp.tile([128, 8, OUT], BF16)  # [hid%128, hh, j2]
            nc.scalar.dma_start(w2t[:], w2[:].rearrange("(hh p) o -> p hh o", p=128))
            b1t = constp.tile([128, 8], F32)
            nc.scalar.dma_start(b1t[:], b1T[:])
            b2t = constp.tile([128, 2], F32)
            nc.scalar.dma_start(b2t[:], b2T[:])
            # x streams per k-block (re-read in each half) -- cheaper than
            # keeping all 8MB resident; the freed SBUF holds more adj cache
            xP_r = xP[:].rearrange("(kb p) (kk j) -> p kb kk j", p=128, kk=2)

            # results kept in SBUF
            yT = constp.tile([128, 4, SH], BF16)  # [j%128, jj, i] = N*y
            hT = constp.tile([128, 8, SH], BF16)  # [hid%128, hh, i]

            # AllGather staging: chunk q covers local rows [512q, 512q+512)
            # laid out [p, kbl, kk, j2] (row = kbl*256 + kk*128 + p).
            ag_in = [dram.tile([128, 1024], FP8, name=f"agi{q}") for q in range(4)]
            ag_out = [
                dram.tile([NCORES * 128, 1024], FP8, addr_space="Shared", name=f"ago{q}")
                for q in range(4)
            ]

            adjU_r = adjU[:].rearrange("(kb p) (kk i) -> p kb kk i", p=128, kk=2)

            # P2 is DMA-bandwidth-bound: keep adj k-blocks loaded during P1
            # resident in SBUF so P2 skips their reload (saves 10MB of the
            # ~36MB P2 stream).  Spread across the AG-arrival groups.
            CACHE_KBS = tuple(range(0, 40, 2))
            adj_cache = {}

            for H in range(2):
                # ---- P1 half H: psY[j, i-1024-half] += x8^T adjC8 ----
                psY = [
                    psum.tile([128, 512], F32, tag="ps", name=f"psY{H}{t}")
                    for t in range(8)
                ]  # tile t = (jj, b): jj*2 + b; holds i-chunks 2b, 2b+1
                for kb in range(64):
                    xt = smallp.tile([128, 2, F], FP8, tag="xt", bufs=8)
                    nc.sync.dma_start(xt[:], xP_r[:, kb])
                    if kb in CACHE_KBS:
                        at = constp.tile([128, 2, 1024], FP8, name=f"ac_{H}_{kb}")
                        adj_cache[(H, kb)] = at
                    else:
                        at = adjp.tile(
                            [128, 2, 1024], FP8, tag="adjt", name=f"a1_{H}_{kb}"
                        )
                    nc.sync.dma_start(
                        at[:], adjU_r[:, kb, :, H * 1024 : (H + 1) * 1024]
                    )
                    for jj in range(4):
                        lhs = xt[:, :, jj * 128 : (jj + 1) * 128]
                        for c in range(4):  # i-chunk of 256 within the half
                            nc.tensor.matmul(
                                psY[jj * 2 + c // 2][:, (c % 2) * 256 : (c % 2) * 256 + 256],
                                lhs,
                                at[:, :, c * 256 : (c + 1) * 256],
                                start=(kb == 0 and c % 2 == 0),
                                stop=(kb == 63 and c % 2 == 1),
                                perf_mode=DR,
                            )
                # drain psY -> yT (bf16) split DVE/ACT: the supportT matmuls
                # contract over all jj so they wait on the LAST drain, and the
                # ACT queue is idle at the half boundary
                for jj in range(4):
                    for b in range(2):
                        dst = yT[:, jj, H * 1024 + b * 512 : H * 1024 + b * 512 + 512]
                        if b == 0:
                            nc.vector.tensor_copy(dst, psY[jj * 2 + b][:])
                        else:
                            nc.scalar.activation(
                                dst,
                                psY[jj * 2 + b][:],
                                mybir.ActivationFunctionType.Copy,
                            )

                # ---- supportT + relu: hT = relu(W1n^T yT + b1) ----
                # hh-groups of 4 with i-width 1024: each stationary W1 block
                # feeds two 512-wide matmuls, halving LDWEIGHTS count.
                for hg in range(2):
                    i0 = H * 1024
                    psS = [
                        psum.tile([128, 512], F32, tag="ps", name=f"psS{H}{hg}{t}")
                        for t in range(8)
                    ]  # t = hh4*2 + qq
                    for hh4 in range(4):
                        hh = hg * 4 + hh4
                        for jj in range(4):
                            for qq in range(2):
                                nc.tensor.matmul(
                                    psS[hh4 * 2 + qq][:],
                                    w1t[:, jj, hh * 128 : (hh + 1) * 128],
                                    yT[:, jj, i0 + qq * 512 : i0 + qq * 512 + 512],
                                    start=(jj == 0),
                                    stop=(jj == 3),
                                )
                    # relu drains split ACT/DVE: the first s2 matmul group is
                    # paced by these, so halve the chain latency
                    for hh4 in range(4):
                        hh = hg * 4 + hh4
                        for qq in range(2):
                            dst = hT[:, hh, i0 + qq * 512 : i0 + qq * 512 + 512]
                            src = psS[hh4 * 2 + qq][:]
                            if qq == 1:
                                nc.vector.tensor_scalar(
                                    dst,
                                    src,
                                    b1t[:, hh : hh + 1],
                                    0.0,
                                    op0=mybir.AluOpType.add,
                                    op1=mybir.AluOpType.max,
                                )
                            else:
                                nc.scalar.activation(
                                    dst,
                                    src,
                                    mybir.ActivationFunctionType.Relu,
                                    bias=b1t[:, hh : hh + 1],
                                )

                # ---- s2 = h @ W2, quantized fp8*256, staged for AG ----
                # psum/AG column order (j2t, kk, jp) so P2's stationary load
                # is a plain contiguous copy.
                for qq in range(2):
                    q = H * 2 + qq  # global chunk id
                    for kbl in range(2):
                        ps2 = psum.tile(
                            [128, 2, 2, 128], F32, tag="ps", name=f"ps2{q}{kbl}"
                        )
                        for kk in range(2):
                            i0 = q * 512 + kbl * 256 + kk * 128
                            for hh in range(8):
                                nc.tensor.matmul(
                                    ps2[:, :, kk, :],
                                    hT[:, hh, i0 : i0 + 128],
                                    w2t[:, hh, :],
                                    start=(hh == 0 and kk == 0),
                                    stop=(hh == 7 and kk == 1),
                                )
                        # drains alternate ACT/DVE and stores go on gpsimd:
                        # these COPYs release the psum banks the next phase's
                        # matmuls reuse, so their chain latency is exposed
                        s2q = smallp.tile([128, 512], FP8, tag="s2q", bufs=4)
                        if kbl == 0:
                            nc.scalar.activation(
                                s2q[:],
                                ps2[:].rearrange("p a b j -> p (a b j)"),
                                mybir.ActivationFunctionType.Copy,
                                scale=S2SCALE,
                            )
                        else:
                            nc.vector.tensor_scalar(
                                s2q[:],
                                ps2[:].rearrange("p a b j -> p (a b j)"),
                                S2SCALE,
                                None,
                                op0=mybir.AluOpType.mult,
                            )
                        nc.gpsimd.dma_start(
                            ag_in[q][:, kbl * 512 : kbl * 512 + 512], s2q[:]
                        )
                    allgather(ag_in[q], ag_out[q])

            # ---- P2: out2T = (adjU8 @ s2q)^T / (N*256) + b2 ----
            psD = [
                psum.tile([128, 512], F32, tag="ps", name=f"psD{t}") for t in range(8)
            ]  # tile t = (j2, cb): j2*4 + cb; holds i-chunks 2cb, 2cb+1
            kb_order = [
                8 * c + 2 * q + t for q in range(4) for c in range(NCORES) for t in range(2)
            ]
            for ki, kb in enumerate(kb_order):
                c, rem = kb // 8, kb % 8
                q, t = rem // 2, rem % 2
                if kb in CACHE_KBS:
                    at = None  # rhs comes from the SBUF-cached P1 tiles
                else:
                    at = adjp.tile([128, 2, 2048], FP8, tag="adjt", name=f"a2_{kb}")
                    nc.sync.dma_start(at[:], adjU_r[:, kb])
                st = smallp.tile([128, 2, 2, 128], FP8, tag="st", bufs=4, name=f"st{kb}")
                # ag_out rows c*128+p, cols t*512 + j2t*256 + kk*128 + jp
                nc.sync.dma_start(
                    st[:].rearrange("p a b j -> p (a b j)"),
                    ag_out[q][c * 128 : (c + 1) * 128, t * 512 : (t + 1) * 512],
                )
                for j2 in range(2):
                    lhs = st[:, j2]
                    for c8 in range(8):
                        if at is not None:
                            rhs = at[:, :, c8 * 256 : (c8 + 1) * 256]
                        else:
                            cc = c8 % 4
                            rhs = adj_cache[(c8 // 4, kb)][
                                :, :, cc * 256 : (cc + 1) * 256
                            ]
                        nc.tensor.matmul(
                            psD[j2 * 4 + c8 // 2][:, (c8 % 2) * 256 : (c8 % 2) * 256 + 256],
                            lhs,
                            rhs,
                            start=(ki == 0 and c8 % 2 == 0),
                            stop=(ki == 63 and c8 % 2 == 1),
                            perf_mode=DR,
                        )
            # final drain split across ACT and DVE so the tail is ~2x shorter
            for j2 in range(2):
                for cb in range(4):
                    ot = smallp.tile([128, 512], F16, tag="ot", bufs=4)
                    if cb % 2 == 0:
                        nc.scalar.activation(
                            ot[:],
                            psD[j2 * 4 + cb][:],
                            mybir.ActivationFunctionType.Identity,
                            bias=b2t[:, j2 : j2 + 1],
                            scale=1.0 / (N * S2SCALE),
                        )
                        nc.scalar.dma_start(
                            out2T[j2 * 128 : (j2 + 1) * 128, cb * 512 : (cb + 1) * 512],
                            ot[:],
                        )
                    else:
                        nc.vector.tensor_scalar(
                            ot[:],
                            psD[j2 * 4 + cb][:],
                            1.0 / (N * S2SCALE),
                            b2t[:, j2 : j2 + 1],
                            op0=mybir.AluOpType.mult,
                            op1=mybir.AluOpType.add,
                        )
                        nc.gpsimd.dma_start(
                            out2T[j2 * 128 : (j2 + 1) * 128, cb * 512 : (cb + 1) * 512],
                            ot[:],
                        )

    _elide_redundant_ldweights(nc)
    _split_excess_waits(nc)
    return nc


def _prep_inputs(x, adj, W1, b1, W2, b2):
    bf = ml_dtypes.bfloat16
    f8 = ml_dtypes.float8_e4m3fn

    u = adj * np.float32(N)  # exact: adj was u/N with N a power of two
    u8 = u.astype(f8)
    x8 = x.astype(f8)
    b1T = np.ascontiguousarray(b1.reshape(HID // 128, 128).T).astype(np.float32)
    b2T = np.ascontiguousarray(b2.reshape(OUT // 128, 128).T).astype(np.float32)
    w1n = (W1 / np.float32(N)).astype(bf)
    w2b = W2.astype(bf)
    # xP[kb*128+p, kk*512+j] = x8[kb*256+kk*128+p, j]
    xP = np.ascontiguousarray(
        x8.reshape(64, 2, 128, F).transpose(0, 2, 1, 3).reshape(8192, 2 * F)
    )

    def adj_layout(a8, rows):
        # out[kb*128+p, kk*2048+i] = a8[rows][i, kb*256+kk*128+p]
        blk = a8[rows, :].reshape(SH, 64, 2, 128)  # [i, kb, kk, p]
        return np.ascontiguousarray(
            blk.transpose(1, 3, 2, 0).reshape(8192, 2 * SH)
        )

    in_maps = []
    for c in range(NCORES):
        rows = slice(c * SH, (c + 1) * SH)
        in_maps.append(
            {
                "adjU": adj_layout(u8, rows),
                "xP": xP,
                "w1n": w1n,
                "w2": w2b,
                "b1T": b1T,
                "b2T": b2T,
            }
        )
    return in_maps


def _run(inputs, trace=False):
    global _built
    if _built is None:
        _built = build()
    in_maps = _prep_inputs(**inputs)
    r = run_bass_kernel_spmd(_built, in_maps, list(range(NCORES)), trace=trace)
    out = np.empty([N, OUT], np.float32)
    for c in range(NCORES):
        out[c * SH : (c + 1) * SH, :] = r.results[c]["out2T"].T.astype(np.float32)
    return out, r


def kernel(x, adj, W1, b1, W2, b2):
    out, _ = _run(dict(x=x, adj=adj, W1=W1, b1=b1, W2=W2, b2=b2))
    return out


# revision 63
# speedup vs baseline: 1.0282x; 1.0086x over previous
"""Trainium2 Bass kernel for a 2-layer dense GCN (NodeEncoder).

    out = adj @ relu(adj @ (x@W1) + b1) @ W2 + b2
    N=16384, F_IN=512, HID=1024, OUT=256, adj dense [N, N] fp32.

Algorithm (reassociated to nearly halve layer-1 FLOPs and drop the big
s1 AllGather):  relu(adj @ (x@W1)) == relu((adj@x) @ W1), so per core
(adj row-partitioned, 2048 rows each):

  P1:    yT_c   = x8^T @ adjN8_c^T                     [512, 2048]  (= N*y^T)
  small: hT_c   = relu(yT_c^T @ (W1/N) + b1)^T         [1024, 2048] bf16
         s2_c   = h_c @ W2                             [2048, 256]
         quantized to fp8 * 256 for the gather.
  AG:    s2q    = AllGather(s2q_c)  (4 chunks of 128KB, overlapped)
  P2:    out2T_c = (adjN8_c @ s2q)^T / (N*256) + b2    [256, 2048] fp32

Big matmuls run in fp8-e4m3 DoubleRow (K=256/instr); small ones bf16.
Simulated end-to-end rel err ~1.55e-2 vs fp32 reference (tol 2e-2),
dominated by the fp8 quantization of x.
"""

import numpy as np
import ml_dtypes

import concourse.bass as bass
import concourse.mybir as mybir
import concourse.tile as tile
from concourse.bass_utils import run_bass_kernel_spmd
from concourse.tile_sem_assignment import N_PROCS
from concourse.vector_clock import ScopedClock, VectorClock

# ---------------------------------------------------------------------------
# Workaround: the walrus build in this container caps the number of sync-wait
# commands per instruction at ONE.  Tile's kernel-tail drain aggregates one
# wait per logical processor; split it into a chain of single-wait drains.
# Excess waits on regular instructions are hoisted onto no-ops.
# ---------------------------------------------------------------------------


def _drain_and_barrier_split(self, tick_clock, wait_clock):
    gc = tick_clock.global_clock
    for p in range(N_PROCS):
        partial = VectorClock([gc[q] if q == p else 0 for q in range(N_PROCS)])
        d = self.nc.sync.nop(nofuse=True)
        wait_clock.add_sem_waits(d.ins, ScopedClock({None: partial}))
    self.nc.sync.drain()

    self.nc.all_engine_barrier()
    assert self.sems is not None
    popped = self.nc._tile_sem_poison_stack.pop()
    assert popped is self._sem_poison
    self.nc.clear_and_free_semaphores(list(self.sems.allocated().values()))
    self.nc.all_engine_barrier()


tile.TileContext._drain_and_barrier = _drain_and_barrier_split

_MAX_WAITS = 1


def _split_excess_waits(nc):
    ctr = 0
    for f in nc.m.functions:
        for bb in f.blocks:
            out = []
            changed = False
            for inst in bb.instructions:
                si = inst.sync_info
                waits = list(si.on_wait) if si is not None and si.on_wait else []
                if len(waits) > _MAX_WAITS:
                    changed = True
                    keep, excess = waits[: _MAX_WAITS], waits[_MAX_WAITS :]
                    for i in range(0, len(excess), _MAX_WAITS):
                        ctr += 1
                        nop = mybir.InstNoOp(name=f"I-waitnop-{ctr}")
                        nop.engine = inst.engine
                        nop.sync_info = mybir.SyncInfo(
                            on_wait=excess[i : i + _MAX_WAITS], on_update=[]
                        )
                        out.append(nop)
                    si.on_wait = keep
                out.append(inst)
            if changed:
                bb.instructions = out
    return ctr


def _elide_redundant_ldweights(nc):
    """Drop an InstLdweights that reloads the same weights AP as the previous
    surviving one with only plain matmuls/no-ops in between (the PE keeps the
    stationary operand across matmuls; walrus emits one LDWEIGHTS per MATMUL)."""
    n_elided = 0
    for f in nc.m.functions:
        for bb in f.blocks:
            out = []
            last_w = None
            changed = False
            for inst in bb.instructions:
                nm = type(inst).__name__
                if nm == "InstLdweights":
                    si = inst.sync_info
                    clean = not (si and (si.on_wait or si.on_update))
                    w = repr(inst.ins[0])
                    if clean and last_w == w:
                        n_elided += 1
                        changed = True
                        continue
                    last_w = w if clean else None
                elif nm == "InstMatmult":
                    if getattr(inst, "is_transpose", False):
                        last_w = None
                elif nm == "InstNoOp":
                    pass
                else:
                    last_w = None
                out.append(inst)
            if changed:
                bb.instructions = out
    return n_elided


NCORES = 8
N = 16384
SH = N // NCORES  # 2048 adj rows per core
F = 512
HID = 1024
OUT = 256
S2SCALE = 256.0  # s2 is gathered as fp8 of 256*s2

BF16 = mybir.dt.bfloat16
F16 = mybir.dt.float16
F32 = mybir.dt.float32
FP8 = mybir.dt.float8e4
DR = mybir.MatmulPerfMode.DoubleRow

_built = None


def build():
    nc = bass.Bass()

    # adjU row r = kb*128 + p (k = kb*256 + kk*128 + p global col of adjT_c),
    # col = kk*2048 + i (i = local row of the adj shard), values N*adj in fp8.
    # P1 reads the 1024-wide i-half slices, P2 reads full rows.
    adjU = nc.declare_dram_parameter("adjU", [8192, 2 * SH], FP8, isOutput=False)
    # xP row = kb*128 + p, col = kk*512 + j
    xP = nc.declare_dram_parameter("xP", [8192, 2 * F], FP8, isOutput=False)
    w1n = nc.declare_dram_parameter("w1n", [F, HID], BF16, isOutput=False)  # W1/N
    w2 = nc.declare_dram_parameter("w2", [HID, OUT], BF16, isOutput=False)
    b1T = nc.declare_dram_parameter("b1T", [128, HID // 128], F32, isOutput=False)
    b2T = nc.declare_dram_parameter("b2T", [128, OUT // 128], F32, isOutput=False)
    out2T = nc.declare_dram_parameter("out2T", [OUT, SH], F16, isOutput=True)

    rg = [list(range(NCORES))]

    def allgather(inp, outp):
        return nc.gpsimd.collective_compute(
            "AllGather",
            mybir.AluOpType.bypass,
            replica_groups=rg,
            ins=[inp.opt()],
            outs=[outp.opt()],
        )

    with tile.TileContext(nc) as tc:
        with (
            tc.tile_pool(name="const", bufs=1) as constp,
            tc.tile_pool(name="psum", bufs=8, space="PSUM") as psum,
            tc.tile_pool(name="dram", bufs=1, space="DRAM") as dram,
            tc.tile_pool(name="adj", bufs=6) as adjp,
            tc.tile_pool(name="small", bufs=4) as smallp,
        ):
            # ---- constants / resident tensors ----
            # consts go on the ACT dma queue so the SP queue starts with the
            # x/adj tiles that gate the first matmul
            w1t = constp.tile([128, 4, HID], BF16)  # [j%128, jj, hid]
            nc.scalar.dma_start(w1t[:], w1n[:].rearrange("(jj p) h -> p jj h", p=128))
            w2t = constp.tile([128, 8, OUT], BF16)  # [hid%128, hh, j2]
            nc.scalar.dma_start(w2t[:], w2[:].rearrange("(hh p) o -> p hh o", p=128))
            b1t = constp.tile([128, 8], F32)
            nc.scalar.dma_start(b1t[:], b1T[:])
            b2t = constp.tile([128, 2], F32)
            nc.scalar.dma_start(b2t[:], b2T[:])
            # x streams per k-block (re-read in each half) -- cheaper than
            # keeping all 8MB resident; the freed SBUF holds more adj cache
            xP_r = xP[:].rearrange("(kb p) (kk j) -> p kb kk j", p=128, kk=2)

            # results kept in SBUF
            yT = constp.tile([128, 4, SH], BF16)  # [j%128, jj, i] = N*y
            hT = constp.tile([128, 8, SH], BF16)  # [hid%128, hh, i]

            # AllGather staging: chunk q covers local rows [512q, 512q+512)
            # laid out [p, kbl, kk, j2] (row = kbl*256 + kk*128 + p).
            ag_in = [dram.tile([128, 1024], FP8, name=f"agi{q}") for q in range(4)]
            ag_out = [
                dram.tile([NCORES * 128, 1024], FP8, addr_space="Shared", name=f"ago{q}")
                for q in range(4)
            ]

            adjU_r = adjU[:].rearrange("(kb p) (kk i) -> p kb kk i", p=128, kk=2)

            # P2 is DMA-bandwidth-bound: keep adj k-blocks loaded during P1
            # resident in SBUF so P2 skips their reload (saves 10MB of the
            # ~36MB P2 stream).  Spread across the AG-arrival groups.
            CACHE_KBS = tuple(range(0, 40, 2))
            adj_cache = {}

            for H in range(2):
                # ---- P1 half H: psY[j, i-1024-half] += x8^T adjC8 ----
                psY = [
                    psum.tile([128, 512], F32, tag="ps", name=f"psY{H}{t}")
                    for t in range(8)
                ]  # tile t = (jj, b): jj*2 + b; holds i-chunks 2b, 2b+1
                for kb in range(64):
                    xt = smallp.tile([128, 2, F], FP8, tag="xt", bufs=8)
                    nc.sync.dma_start(xt[:], xP_r[:, kb])
                    if kb in CACHE_KBS:
                        at = constp.tile([128, 2, 1024], FP8, name=f"ac_{H}_{kb}")
                        adj_cache[(H, kb)] = at
                    else:
                        at = adjp.tile(
                            [128, 2, 1024], FP8, tag="adjt", name=f"a1_{H}_{kb}"
                        )
                    nc.sync.dma_start(
                        at[:], adjU_r[:, kb, :, H * 1024 : (H + 1) * 1024]
                    )
                    for jj in range(4):
                        lhs = xt[:, :, jj * 128 : (jj + 1) * 128]
                        for c in range(4):  # i-chunk of 256 within the half
                            nc.tensor.matmul(
                                psY[jj * 2 + c // 2][:, (c % 2) * 256 : (c % 2) * 256 + 256],
                                lhs,
                                at[:, :, c * 256 : (c + 1) * 256],
                                start=(kb == 0 and c % 2 == 0),
                                stop=(kb == 63 and c % 2 == 1),
                                perf_mode=DR,
                            )
                # drain psY -> yT (bf16) split DVE/ACT: the supportT matmuls
                # contract over all jj so they wait on the LAST drain, and the
                # ACT queue is idle at the half boundary
                for jj in range(4):
                    for b in range(2):
                        dst = yT[:, jj, H * 1024 + b * 512 : H * 1024 + b * 512 + 512]
                        if b == 0:
                            nc.vector.tensor_copy(dst, psY[jj * 2 + b][:])
                        else:
                            nc.scalar.activation(
                                dst,
                                psY[jj * 2 + b][:],
                                mybir.ActivationFunctionType.Copy,
                            )

                # ---- supportT + relu: hT = relu(W1n^T yT + b1) ----
                # hh-groups of 4 with i-width 1024: each stationary W1 block
                # feeds two 512-wide matmuls, halving LDWEIGHTS count.
                for hg in range(2):
                    i0 = H * 1024
                    psS = [
                        psum.tile([128, 512], F32, tag="ps", name=f"psS{H}{hg}{t}")
                        for t in range(8)
                    ]  # t = hh4*2 + qq
                    for hh4 in range(4):
                        hh = hg * 4 + hh4
                        for jj in range(4):
                            for qq in range(2):
                                nc.tensor.matmul(
                                    psS[hh4 * 2 + qq][:],
                                    w1t[:, jj, hh * 128 : (hh + 1) * 128],
                                    yT[:, jj, i0 + qq * 512 : i0 + qq * 512 + 512],
                                    start=(jj == 0),
                                    stop=(jj == 3),
                                )
                    # relu drains split ACT/DVE: the first s2 matmul group is
                    # paced by these, so halve the chain latency
                    for hh4 in range(4):
                        hh = hg * 4 + hh4
                        for qq in range(2):
                            dst = hT[:, hh, i0 + qq * 512 : i0 + qq * 512 + 512]
                            src = psS[hh4 * 2 + qq][:]
                            if qq == 1:
                                nc.vector.tensor_scalar(
                                    dst,
                                    src,
                                    b1t[:, hh : hh + 1],
                                    0.0,
                                    op0=mybir.AluOpType.add,
                                    op1=mybir.AluOpType.max,
                                )
                            else:
                                nc.scalar.activation(
                                    dst,
                                    src,
                                    mybir.ActivationFunctionType.Relu,
                                    bias=b1t[:, hh : hh + 1],
                                )

                # ---- s2 = h @ W2, quantized fp8*256, staged for AG ----
                # psum/AG column order (j2t, kk, jp) so P2's stationary load
                # is a plain contiguous copy.
                for qq in range(2):
                    q = H * 2 + qq  # global chunk id
                    for kbl in range(2):
                        ps2 = psum.tile(
                            [128, 2, 2, 128], F32, tag="ps", name=f"ps2{q}{kbl}"
                        )
                        for kk in range(2):
                            i0 = q * 512 + kbl * 256 + kk * 128
                            for hh in range(8):
                                nc.tensor.matmul(
                                    ps2[:, :, kk, :],
                                    hT[:, hh, i0 : i0 + 128],
                                    w2t[:, hh, :],
                                    start=(hh == 0 and kk == 0),
                                    stop=(hh == 7 and kk == 1),
                                )
                        # drains alternate ACT/DVE and stores go on gpsimd:
                        # these COPYs release the psum banks the next phase's
                        # matmuls reuse, so their chain latency is exposed
                        s2q = smallp.tile([128, 512], FP8, tag="s2q", bufs=4)
                        if kbl == 0:
                            nc.scalar.activation(
                                s2q[:],
                                ps2[:].rearrange("p a b j -> p (a b j)"),
                                mybir.ActivationFunctionType.Copy,
                                scale=S2SCALE,
                            )
                        else:
                            nc.vector.tensor_scalar(
                                s2q[:],
                                ps2[:].rearrange("p a b j -> p (a b j)"),
                                S2SCALE,
                                None,
                                op0=mybir.AluOpType.mult,
                            )
                        nc.gpsimd.dma_start(
                            ag_in[q][:, kbl * 512 : kbl * 512 + 512], s2q[:]
                        )
                    allgather(ag_in[q], ag_out[q])

            # ---- P2: out2T = (adjU8 @ s2q)^T / (N*256) + b2 ----
            psD = [
                psum.tile([128, 512], F32, tag="ps", name=f"psD{t}") for t in range(8)
            ]  # tile t = (j2, cb): j2*4 + cb; holds i-chunks 2cb, 2cb+1
            # 4 SBUF-cached k-blocks form a bank-staggered tail: emitted
            # bank-by-bank so each psum bank stops ~1us apart and its
            # drain+store pipelines behind the remaining banks' matmuls
            TAIL_KBS = [14, 22, 30, 38]
            kb_order = [
                8 * c + 2 * q + t for q in range(4) for c in range(NCORES) for t in range(2)
            ]
            kb_order = [kb for kb in kb_order if kb not in TAIL_KBS]
            for ki, kb in enumerate(kb_order):
                c, rem = kb // 8, kb % 8
                q, t = rem // 2, rem % 2
                if kb in CACHE_KBS:
                    at = None  # rhs comes from the SBUF-cached P1 tiles
                else:
                    at = adjp.tile([128, 2, 2048], FP8, tag="adjt", name=f"a2_{kb}")
                    nc.sync.dma_start(at[:], adjU_r[:, kb])
                st = smallp.tile([128, 2, 2, 128], FP8, tag="st", bufs=4, name=f"st{kb}")
                # ag_out rows c*128+p, cols t*512 + j2t*256 + kk*128 + jp
                nc.sync.dma_start(
                    st[:].rearrange("p a b j -> p (a b j)"),
                    ag_out[q][c * 128 : (c + 1) * 128, t * 512 : (t + 1) * 512],
                )
                for j2 in range(2):
                    lhs = st[:, j2]
                    for c8 in range(8):
                        if at is not None:
                            rhs = at[:, :, c8 * 256 : (c8 + 1) * 256]
                        else:
                            cc = c8 % 4
                            rhs = adj_cache[(c8 // 4, kb)][
                                :, :, cc * 256 : (cc + 1) * 256
                            ]
                        nc.tensor.matmul(
                            psD[j2 * 4 + c8 // 2][:, (c8 % 2) * 256 : (c8 % 2) * 256 + 256],
                            lhs,
                            rhs,
                            start=(ki == 0 and c8 % 2 == 0),
                            stop=False,
                            perf_mode=DR,
                        )
            # tail: per-bank MMs over the resident k-blocks, then that bank's
            # drain+store immediately -- all pipelined behind later banks' MMs
            st_tail = {}
            for kb in TAIL_KBS:
                c, rem = kb // 8, kb % 8
                q, t = rem // 2, rem % 2
                stt = smallp.tile(
                    [128, 2, 2, 128], FP8, tag="st", bufs=4, name=f"stT{kb}"
                )
                nc.sync.dma_start(
                    stt[:].rearrange("p a b j -> p (a b j)"),
                    ag_out[q][c * 128 : (c + 1) * 128, t * 512 : (t + 1) * 512],
                )
                st_tail[kb] = stt
            for t8 in range(8):
                j2, cb = t8 // 4, t8 % 4
                for n, kb in enumerate(TAIL_KBS):
                    lhs = st_tail[kb][:, j2]
                    for c8 in (2 * cb, 2 * cb + 1):
                        cc = c8 % 4
                        rhs = adj_cache[(c8 // 4, kb)][:, :, cc * 256 : (cc + 1) * 256]
                        nc.tensor.matmul(
                            psD[t8][:, (c8 % 2) * 256 : (c8 % 2) * 256 + 256],
                            lhs,
                            rhs,
                            start=False,
                            stop=(n == len(TAIL_KBS) - 1 and c8 % 2 == 1),
                            perf_mode=DR,
                        )
                ot = smallp.tile([128, 512], F16, tag="ot", bufs=4)
                if cb % 2 == 0:
                    nc.scalar.activation(
                        ot[:],
                        psD[t8][:],
                        mybir.ActivationFunctionType.Identity,
                        bias=b2t[:, j2 : j2 + 1],
                        scale=1.0 / (N * S2SCALE),
                    )
                    nc.scalar.dma_start(
                        out2T[j2 * 128 : (j2 + 1) * 128, cb * 512 : (cb + 1) * 512],
                        ot[:],
                    )
                else:
                    nc.vector.tensor_scalar(
                        ot[:],
                        psD[t8][:],
                        1.0 / (N * S2SCALE),
                        b2t[:, j2 : j2 + 1],
                        op0=mybir.AluOpType.mult,
                        op1=mybir.AluOpType.add,
                    )
                    nc.gpsimd.dma_start(
                        out2T[j2 * 128 : (j2 + 1) * 128, cb * 512 : (cb + 1) * 512],
                        ot[:],
                    )

    _elide_redundant_ldweights(nc)
    _split_excess_waits(nc)
    return nc


def _prep_inputs(x, adj, W1, b1, W2, b2):
    bf = ml_dtypes.bfloat16
    f8 = ml_dtypes.float8_e4m3fn

    u = adj * np.float32(N)  # exact: adj was u/N with N a power of two
    u8 = u.astype(f8)
    x8 = x.astype(f8)
    b1T = np.ascontiguousarray(b1.reshape(HID // 128, 128).T).astype(np.float32)
    b2T = np.ascontiguousarray(b2.reshape(OUT // 128, 128).T).astype(np.float32)
    w1n = (W1 / np.float32(N)).astype(bf)
    w2b = W2.astype(bf)
    # xP[kb*128+p, kk*512+j] = x8[kb*256+kk*128+p, j]
    xP = np.ascontiguousarray(
        x8.reshape(64, 2, 128, F).transpose(0, 2, 1, 3).reshape(8192, 2 * F)
    )

    def adj_layout(a8, rows):
        # out[kb*128+p, kk*2048+i] = a8[rows][i, kb*256+kk*128+p]
        blk = a8[rows, :].reshape(SH, 64, 2, 128)  # [i, kb, kk, p]
        return np.ascontiguousarray(
            blk.transpose(1, 3, 2, 0).reshape(8192, 2 * SH)
        )

    in_maps = []
    for c in range(NCORES):
        rows = slice(c * SH, (c + 1) * SH)
        in_maps.append(
            {
                "adjU": adj_layout(u8, rows),
                "xP": xP,
                "w1n": w1n,
                "w2": w2b,
                "b1T": b1T,
                "b2T": b2T,
            }
        )
    return in_maps


def _run(inputs, trace=False):
    global _built
    if _built is None:
        _built = build()
    in_maps = _prep_inputs(**inputs)
    r = run_bass_kernel_spmd(_built, in_maps, list(range(NCORES)), trace=trace)
    out = np.empty([N, OUT], np.float32)
    for c in range(NCORES):
        out[c * SH : (c + 1) * SH, :] = r.results[c]["out2T"].T.astype(np.float32)
    return out, r


def kernel(x, adj, W1, b1, W2, b2):
    out, _ = _run(dict(x=x, adj=adj, W1=W1, b1=b1, W2=W2, b2=b2))
    return out
